# revision 4
# baseline (speedup 1.0000x reference)
"""AdaptiveESN Trainium2 kernel — sequence-split edition.

Echo State Network: B=64, T=2048, D=128, H=512, leaky a=0.26.
    h_t = (1-a) h_{t-1} + a tanh(x_t W_in^T + b_in + h_{t-1} W_res^T + b_res)
    y_t = h_t W_ro^T

The map is strongly contracting (state error decays ~0.74x/step: leak 0.74,
spectral radius 0.29, tanh saturated), so a segment restarted from h=0
converges to the true trajectory: 32 washout steps give ~6e-6 output error.

Strategy: T=2048 -> 16 segments of 128 steps (+32 washout). Each core runs
2 independent chains (segments 2c, 2c+1), each 64 lanes wide (full batch),
interleaved round-robin so one chain's matmuls hide the other's
ACT/DVE/semaphore epilogue latency. Per chain-step: 20 matmuls of 64 cols
(vs 8 cols in the old data-parallel layout -> 8x fewer PE instructions),
tanh on ACT, one fused AXPY blend on DVE (h' = (1-a) h + p). State h~ = h/a
keeps the leak a single AXPY (a folded into W_res/W_ro). Readout matmuls are
spread one 8-step window at a time between scan rounds.

Layouts (host-prepped, per core c; chain k covers segment s=2c+k, i.e.
global steps t0+r, t0 = 128 s - 32, r in [0,160)):
    xt   bf16 [128, 2*160*64]  xt[d, (k*160+r)*64+b] = x[b, t0+r, d] (0 for t<0)
    wres bf16 [128, 2048]      tile (j,i) at cols (j*4+i)*128: (a W_res).T block
    win  bf16 [128, 512]       W_in.T
    wro  bf16 [128, 512]       tile j at cols j*128: (a W_ro).T block
    bias f32  [128, 16]        col i*4+k*2+ph: (b_in+b_res) chunk i
                               (core 0, k=0, ph=0 -> 0: keeps h=0 in the
                               fake washout of segment 0)
    out  f32  [128, 2*128*64]  out[d, (k*128+u)*64+b] = y[b, 128*s+u, d]
"""
import sys

if "/opt/trn_rl_repo" not in sys.path:
    sys.path.insert(0, "/opt/trn_rl_repo")

import numpy as np
import ml_dtypes

import concourse.bass as bass
from concourse import bacc
import concourse.mybir as mybir
import concourse.tile as tile
from concourse.bass_utils import run_bass_kernel_spmd

try:
    import jax

    jax.config.update("jax_compilation_cache_dir", "/tmp/jax_neff_cache")
    jax.config.update("jax_persistent_cache_min_compile_time_secs", 10)
except Exception:
    pass

B, T, D, H = 64, 2048, 128, 512
LEAKY = 0.26
NCORES = 8
NCH = H // 128            # H chunks (partition tiles)
KPC = 2                   # chains (segments) per core
NSEG = NCORES * KPC       # global segments
SEGLEN = T // NSEG        # useful steps per segment
WO = 32                   # washout steps
STEPS = SEGLEN + WO       # chain length
LW = B                    # lanes per chain (full batch)
WST = NCH * LW            # state cols per step
TCB = 32                  # steps per state buffer
NBUF = 3                  # state buffers per chain
ROW = 8                   # steps per readout window (8*64 = 512 cols)
BF16 = mybir.dt.bfloat16
F32 = mybir.dt.float32

TRACE = False
_last_results = None


def build(t_total=T, tc=TCB, reps=1, probe=None):
    """Build the per-core Bacc graph (same graph on all 8 cores).

    reps > 1 wraps the scan in a hardware For_i loop for wall-clock delta
    timing (per-scan = (wall_hi - wall_lo) / (reps_hi - reps_lo)).

    probe: timing-only structural variants (WRONG math, never for output):
      "zrhs" - scan matmuls read h0 (zero) instead of hprev: no cross-step
               dependency chain; measures pure engine throughput
      "noro" - skip readout matmuls/copies/DMAs
      "nodve" - ACT writes states directly (no blend): removes DVE from chain
    """
    assert t_total == T, "sequence-split layout is hardcoded for T=2048"
    nc = bacc.Bacc(None, target_bir_lowering=False)
    xt_e = nc.declare_dram_parameter("xt", [128, KPC * STEPS * LW], BF16, isOutput=False)
    wres_e = nc.declare_dram_parameter("wres", [128, 16 * 128], BF16, isOutput=False)
    win_e = nc.declare_dram_parameter("win", [128, NCH * 128], BF16, isOutput=False)
    wro_e = nc.declare_dram_parameter("wro", [128, NCH * 128], BF16, isOutput=False)
    bias_e = nc.declare_dram_parameter("bias", [128, NCH * KPC * 2], F32, isOutput=False)
    out_e = nc.declare_dram_parameter("out", [128, KPC * SEGLEN * LW], F32, isOutput=True)

    with tile.TileContext(nc) as tc_ctx:
        with (
            tc_ctx.tile_pool(name="const", bufs=1) as const_pool,
            tc_ctx.tile_pool(name="p", bufs=10) as p_pool,
            tc_ctx.tile_pool(name="ostage", bufs=3) as o_pool,
            tc_ctx.tile_pool(name="scan_ps", bufs=6, space=bass.MemorySpace.PSUM) as ps_pool,
            tc_ctx.tile_pool(name="ro_ps", bufs=2, space=bass.MemorySpace.PSUM) as ro_pool,
        ):
            xt_sb = const_pool.tile([128, KPC * STEPS * LW], BF16)
            wres_sb = const_pool.tile([128, 16 * 128], BF16)
            win_sb = const_pool.tile([128, NCH * 128], BF16)
            wro_sb = const_pool.tile([128, NCH * 128], BF16)
            bias_sb = const_pool.tile([128, NCH * KPC * 2], F32)
            h0_sb = const_pool.tile([128, WST], BF16)
            # states per chain, step-major: col (r%TCB)*WST + i*LW + b
            st = [
                [
                    const_pool.tile([128, TCB * WST], BF16, name=f"st{k}_{n}", tag=f"st{k}_{n}")
                    for n in range(NBUF)
                ]
                for k in range(KPC)
            ]

            nc.sync.dma_start(wres_sb[:], wres_e[:])
            nc.sync.dma_start(win_sb[:], win_e[:])
            nc.sync.dma_start(wro_sb[:], wro_e[:])
            nc.sync.dma_start(bias_sb[:], bias_e[:])
            nc.sync.dma_start(xt_sb[:], xt_e[:])
            nc.vector.memset(h0_sb[:], 0.0)

            def emit_step(k, r):
                ph = 0 if r < WO else 1
                if r == 0:
                    hprev = h0_sb[:]
                else:
                    bprev = ((r - 1) // TCB) % NBUF
                    sprev = (r - 1) % TCB
                    hprev = st[k][bprev][:, sprev * WST : (sprev + 1) * WST]
                bcur = (r // TCB) % NBUF
                scur = r % TCB
                xcol = xt_sb[:, (k * STEPS + r) * LW : (k * STEPS + r + 1) * LW]
                hsrc = h0_sb[:] if probe == "zrhs" else hprev

                def hcol(j):
                    return hsrc[:, j * LW : (j + 1) * LW]

                ps = ps_pool.tile([128, NCH * LW], F32)
                for i in range(NCH):
                    psw = ps[:, i * LW : (i + 1) * LW]
                    # j-order (0,1,2,win,3): defer the h[3] consumption
                    ops = [
                        (wres_sb[:, (0 * NCH + i) * 128 : (0 * NCH + i + 1) * 128], hcol(0)),
                        (wres_sb[:, (1 * NCH + i) * 128 : (1 * NCH + i + 1) * 128], hcol(1)),
                        (wres_sb[:, (2 * NCH + i) * 128 : (2 * NCH + i + 1) * 128], hcol(2)),
                        (win_sb[:, i * 128 : (i + 1) * 128], xcol),
                        (wres_sb[:, (3 * NCH + i) * 128 : (3 * NCH + i + 1) * 128], hcol(3)),
                    ]
                    for kk, (lhsT, rhs) in enumerate(ops):
                        nc.tensor.matmul(
                            psw, lhsT, rhs,
                            start=(kk == 0), stop=(kk == len(ops) - 1))
                    st_col = st[k][bcur][:, scur * WST + i * LW : scur * WST + (i + 1) * LW]
                    bias_ap = bias_sb[:, i * KPC * 2 + k * 2 + ph : i * KPC * 2 + k * 2 + ph + 1]
                    if probe == "nodve":
                        nc.scalar.activation(
                            st_col, psw, mybir.ActivationFunctionType.Tanh,
                            bias=bias_ap,
                        )
                    else:
                        p_t = p_pool.tile([128, LW], BF16)
                        nc.scalar.activation(
                            p_t[:], psw, mybir.ActivationFunctionType.Tanh,
                            bias=bias_ap,
                        )
                        nc.vector.scalar_tensor_tensor(
                            st_col,
                            hprev[:, i * LW : (i + 1) * LW],
                            1.0 - LEAKY,
                            p_t[:],
                            op0=mybir.AluOpType.mult,
                            op1=mybir.AluOpType.add,
                        )

            def emit_ro(k, rs, alt):
                # readout of chain k states for steps [rs, rs+ROW)
                b = (rs // TCB) % NBUF
                ls = rs % TCB
                st_v = st[k][b].rearrange("p (s w) -> p s w", w=WST)
                rps = ro_pool.tile([128, ROW * LW], F32)
                for j in range(NCH):
                    nc.tensor.matmul(
                        rps[:],
                        wro_sb[:, j * 128 : (j + 1) * 128],
                        st_v[:, ls : ls + ROW, j * LW : (j + 1) * LW],
                        start=(j == 0),
                        stop=(j == NCH - 1),
                    )
                ostage = o_pool.tile([128, ROW * LW], F32)
                if alt:
                    nc.scalar.copy(ostage[:], rps[:])
                else:
                    nc.vector.tensor_copy(ostage[:], rps[:])
                nc.sync.dma_start(
                    out_e[:, (k * SEGLEN + rs - WO) * LW : (k * SEGLEN + rs - WO + ROW) * LW],
                    ostage[:],
                )

            def scan_body(_iv=None):
                emitted = [set() for _ in range(KPC)]
                n_ro = 0
                for r in range(STEPS):
                    for k in range(KPC):
                        emit_step(k, r)
                        if probe == "noro":
                            continue
                        lag = 8 + 4 * k
                        rs = r - lag
                        if rs >= WO and (rs - WO) % ROW == 0:
                            emit_ro(k, rs, n_ro % 2 == 0)
                            emitted[k].add(rs)
                            n_ro += 1
                if probe != "noro":
                    for k in range(KPC):
                        for rs in range(WO, STEPS, ROW):
                            if rs not in emitted[k]:
                                emit_ro(k, rs, n_ro % 2 == 0)
                                n_ro += 1

            if reps == 1:
                scan_body()
            else:
                with tc_ctx.For_i(0, reps, 1) as _i:
                    scan_body(_i)

    nc.compile()
    return nc


def host_prep(x, W_in, b_in, W_res, b_res, W_ro, t_total=T):
    """Produce the per-core in_maps (host-side layout/dtype prep only)."""
    a = np.float32(LEAKY)
    AT = (a * W_res).T.astype(np.float32)                     # [in, out]
    wres = (
        AT.reshape(NCH, 128, NCH, 128).transpose(1, 0, 2, 3).reshape(128, 16 * 128)
    ).astype(ml_dtypes.bfloat16)
    win = W_in.T.astype(ml_dtypes.bfloat16)                   # [128, 512]
    R = (a * W_ro).T.astype(np.float32)                       # [512, 128]
    wro = R.reshape(NCH, 128, 128).transpose(1, 0, 2).reshape(128, NCH * 128).astype(
        ml_dtypes.bfloat16
    )
    bvec = (b_in + b_res).astype(np.float32).reshape(NCH, 128)  # [chunk, 128]

    xpad = np.zeros((B, WO, D), np.float32)
    xext = np.concatenate([xpad, x], axis=1)                  # x at t-index t+WO

    in_maps = []
    for c in range(NCORES):
        xt = np.empty((128, KPC * STEPS * LW), np.float32)
        bias = np.empty((128, NCH * KPC * 2), np.float32)
        for k in range(KPC):
            s = KPC * c + k
            t0 = SEGLEN * s - WO
            xl = xext[:, t0 + WO : t0 + WO + STEPS, :]        # [64, 160, 128]
            xt[:, k * STEPS * LW : (k + 1) * STEPS * LW] = (
                xl.transpose(2, 1, 0).reshape(128, STEPS * LW)
            )
            for i in range(NCH):
                for ph in range(2):
                    col = i * KPC * 2 + k * 2 + ph
                    if c == 0 and k == 0 and ph == 0:
                        bias[:, col] = 0.0
                    else:
                        bias[:, col] = bvec[i]
        in_maps.append({
            "xt": xt.astype(ml_dtypes.bfloat16),
            "wres": wres, "win": win, "wro": wro, "bias": bias,
        })
    return in_maps


_nc_cache = {}


def kernel(x, W_in, b_in, W_res, b_res, W_ro):
    """Full inputs in, full output out ([B, T, D] float32)."""
    global _last_results
    x, W_in, b_in, W_res, b_res, W_ro = (
        np.asarray(t, dtype=np.float32) for t in (x, W_in, b_in, W_res, b_res, W_ro)
    )
    t_total = x.shape[1]
    if t_total not in _nc_cache:
        _nc_cache[t_total] = build(t_total=t_total)
    nc = _nc_cache[t_total]

    in_maps = host_prep(x, W_in, b_in, W_res, b_res, W_ro, t_total=t_total)
    res = run_bass_kernel_spmd(nc, in_maps, list(range(NCORES)), trace=TRACE)
    _last_results = res

    out = np.empty((B, t_total, D), dtype=np.float32)
    for c in range(NCORES):
        oc = res.results[c]["out"]                            # [128, 2*128*64]
        for k in range(KPC):
            s = KPC * c + k
            seg = oc[:, k * SEGLEN * LW : (k + 1) * SEGLEN * LW]
            out[:, s * SEGLEN : (s + 1) * SEGLEN, :] = (
                seg.reshape(128, SEGLEN, LW).transpose(2, 1, 0)
            )
    return out


# revision 6
# speedup vs baseline: 1.2747x; 1.2747x over previous
"""AdaptiveESN Trainium2 kernel — fused sequence-split edition (v3).

Echo State Network: B=64, T=2048, D=128, H=512, leaky a=0.26.
    h_t = (1-a) h_{t-1} + a tanh(x_t W_in^T + b_in + h_{t-1} W_res^T + b_res)
    y_t = h_t W_ro^T

The map is strongly contracting (state error decays ~0.74x/step), so a
chain restarted from h=0 converges to the true trajectory in ~32 steps.

Strategy: 16 overlapping slots of 160 steps. Slot 0 starts at t=0 (h=0 is
exact, all 160 outputs valid); slot s>=1 starts at t=128s-32 and its first
32 outputs are discarded (washout). Core c runs ONE fused 128-lane chain =
slots 2c (lanes 0-63) and 2c+1 (lanes 64-127), batch = all 64 rows. Per
round: 20 matmuls of 128 cols (16 W_res + 4 W_in), 4 tanh on ACT (bias via
per-partition ACT bias), 4 fused AXPY blends on DVE (h' = (1-a) h + p,
state h~ = h/a with a folded into W_res/W_ro). j=3 matmuls are emitted
last and chunk epilogues ordered 0,1,2,3 so the cross-engine recurrence
chain hides under the ~16-matmul early block. Readout (4 steps x 128
lanes per window) is spread one window per 4 rounds.

Layouts (host-prepped, per core c; slot s=2c+half at lanes half*64..):
    xt   bf16 [128, 160*128]  xt[d, r*128+half*64+b] = x[b, t_s+r, d]
    wres bf16 [128, 2048]     tile (j,i) at cols (j*4+i)*128: (a W_res).T block
    win  bf16 [128, 512]      W_in.T
    wro  bf16 [128, 512]      tile j at cols j*128: (a W_ro).T block
    bias f32  [128, 4]        (b_in + b_res) chunk i in col i
    out  f32  [128, 160*128]  out[d, r*128+half*64+b] = y[b, t_s+r, d]
with t_s = 0 for s=0 else 128 s - 32; host keeps steps [0,128) of slot 0
and [32,160) of slots s>=1.
"""
import sys

if "/opt/trn_rl_repo" not in sys.path:
    sys.path.insert(0, "/opt/trn_rl_repo")

import numpy as np
import ml_dtypes

import concourse.bass as bass
from concourse import bacc
import concourse.mybir as mybir
import concourse.tile as tile
from concourse.bass_utils import run_bass_kernel_spmd

try:
    import jax

    jax.config.update("jax_compilation_cache_dir", "/tmp/jax_neff_cache")
    jax.config.update("jax_persistent_cache_min_compile_time_secs", 10)
except Exception:
    pass

B, T, D, H = 64, 2048, 128, 512
LEAKY = 0.26
NCORES = 8
NCH = H // 128            # H chunks (partition tiles)
NSLOT = 16                # global slots, 2 per core
SEGLEN = T // NSLOT       # stride between slot starts
WO = 32                   # washout (discarded) steps for slots >= 1
STEPS = SEGLEN + WO       # chain length per slot
LANES = 128               # fused lanes per core (2 slots x 64 batch)
WST = NCH * LANES         # state cols per step (512)
TCB = 32                  # steps per state buffer
NBUF = 3                  # state buffers
ROW = 4                   # steps per readout window (4*128 = 512 cols)
BF16 = mybir.dt.bfloat16
F32 = mybir.dt.float32

TRACE = False
_last_results = None


def slot_t0(s):
    return 0 if s == 0 else SEGLEN * s - WO


def build(t_total=T, tc=TCB, reps=1, probe=None):
    """Build the per-core Bacc graph (same graph on all 8 cores).

    reps > 1 wraps the scan in a hardware For_i loop for wall-clock delta
    timing (per-scan = (wall_hi - wall_lo) / (reps_hi - reps_lo)).

    probe: timing-only structural variants (WRONG math, never for output):
      "zrhs"  - scan matmuls read h0 (zero) instead of hprev
      "noro"  - skip readout matmuls/copies/DMAs
      "nodve" - ACT writes states directly (no blend)
      "noact" - skip ACT+DVE epilogue entirely (pure matmul/RO rate)
    """
    assert t_total == T, "slot layout is hardcoded for T=2048"
    nc = bacc.Bacc(None, target_bir_lowering=False)
    xt_e = nc.declare_dram_parameter("xt", [128, STEPS * LANES], BF16, isOutput=False)
    wres_e = nc.declare_dram_parameter("wres", [128, 16 * 128], BF16, isOutput=False)
    win_e = nc.declare_dram_parameter("win", [128, NCH * 128], BF16, isOutput=False)
    wro_e = nc.declare_dram_parameter("wro", [128, NCH * 128], BF16, isOutput=False)
    bias_e = nc.declare_dram_parameter("bias", [128, NCH], F32, isOutput=False)
    out_e = nc.declare_dram_parameter("out", [128, STEPS * LANES], F32, isOutput=True)

    with tile.TileContext(nc) as tc_ctx:
        with (
            tc_ctx.tile_pool(name="const", bufs=1) as const_pool,
            tc_ctx.tile_pool(name="p", bufs=8) as p_pool,
            tc_ctx.tile_pool(name="ostage", bufs=3) as o_pool,
            tc_ctx.tile_pool(name="scan_ps", bufs=4, space=bass.MemorySpace.PSUM) as ps_pool,
            tc_ctx.tile_pool(name="ro_ps", bufs=2, space=bass.MemorySpace.PSUM) as ro_pool,
        ):
            xt_sb = const_pool.tile([128, STEPS * LANES], BF16)
            wres_sb = const_pool.tile([128, 16 * 128], BF16)
            win_sb = const_pool.tile([128, NCH * 128], BF16)
            wro_sb = const_pool.tile([128, NCH * 128], BF16)
            bias_sb = const_pool.tile([128, NCH], F32)
            h0_sb = const_pool.tile([128, WST], BF16)
            # states, step-major: col (r%TCB)*WST + i*LANES + lane
            st = [
                const_pool.tile([128, TCB * WST], BF16, name=f"st{n}", tag=f"st{n}")
                for n in range(NBUF)
            ]

            nc.sync.dma_start(wres_sb[:], wres_e[:])
            nc.sync.dma_start(win_sb[:], win_e[:])
            nc.sync.dma_start(wro_sb[:], wro_e[:])
            nc.sync.dma_start(bias_sb[:], bias_e[:])
            nc.sync.dma_start(xt_sb[:], xt_e[:])
            nc.vector.memset(h0_sb[:], 0.0)

            def emit_step(r):
                if r == 0:
                    hprev = h0_sb[:]
                else:
                    bprev = ((r - 1) // TCB) % NBUF
                    sprev = (r - 1) % TCB
                    hprev = st[bprev][:, sprev * WST : (sprev + 1) * WST]
                bcur = (r // TCB) % NBUF
                scur = r % TCB
                xcol = xt_sb[:, r * LANES : (r + 1) * LANES]
                hsrc = h0_sb[:] if probe == "zrhs" else hprev

                ps = ps_pool.tile([128, WST], F32)

                def psw(i):
                    return ps[:, i * LANES : (i + 1) * LANES]

                def hcol(j):
                    return hsrc[:, j * LANES : (j + 1) * LANES]

                # per-region groups kept sequential (interleaving groups on
                # one PSUM tile mis-accumulates); j-order (win,0,1,2,3)
                # defers the h[3] consumption within each group.
                for i in range(NCH):
                    ops = [
                        (win_sb[:, i * 128 : (i + 1) * 128], xcol),
                        (wres_sb[:, (0 * NCH + i) * 128 : (0 * NCH + i + 1) * 128], hcol(0)),
                        (wres_sb[:, (1 * NCH + i) * 128 : (1 * NCH + i + 1) * 128], hcol(1)),
                        (wres_sb[:, (2 * NCH + i) * 128 : (2 * NCH + i + 1) * 128], hcol(2)),
                        (wres_sb[:, (3 * NCH + i) * 128 : (3 * NCH + i + 1) * 128], hcol(3)),
                    ]
                    for kk, (lhsT, rhs) in enumerate(ops):
                        nc.tensor.matmul(
                            psw(i), lhsT, rhs,
                            start=(kk == 0), stop=(kk == len(ops) - 1))
                if probe == "noact":
                    return
                # epilogue: chunks 0-2 are consumed at the START of the next
                # round's early block, chunk 3 only by its late block.
                for i in range(NCH):
                    st_col = st[bcur][:, scur * WST + i * LANES : scur * WST + (i + 1) * LANES]
                    bias_ap = bias_sb[:, i : i + 1]
                    if probe == "nodve":
                        nc.scalar.activation(
                            st_col, psw(i), mybir.ActivationFunctionType.Tanh,
                            bias=bias_ap,
                        )
                    else:
                        p_t = p_pool.tile([128, LANES], BF16)
                        nc.scalar.activation(
                            p_t[:], psw(i), mybir.ActivationFunctionType.Tanh,
                            bias=bias_ap,
                        )
                        nc.vector.scalar_tensor_tensor(
                            st_col,
                            hprev[:, i * LANES : (i + 1) * LANES],
                            1.0 - LEAKY,
                            p_t[:],
                            op0=mybir.AluOpType.mult,
                            op1=mybir.AluOpType.add,
                        )

            def emit_ro(rs, alt):
                # readout of states for steps [rs, rs+ROW), all 128 lanes
                b = (rs // TCB) % NBUF
                ls = rs % TCB
                st_v = st[b].rearrange("p (s w) -> p s w", w=WST)
                rps = ro_pool.tile([128, ROW * LANES], F32)
                for j in range(NCH):
                    nc.tensor.matmul(
                        rps[:],
                        wro_sb[:, j * 128 : (j + 1) * 128],
                        st_v[:, ls : ls + ROW, j * LANES : (j + 1) * LANES],
                        start=(j == 0),
                        stop=(j == NCH - 1),
                    )
                ostage = o_pool.tile([128, ROW * LANES], F32)
                if alt:
                    nc.scalar.copy(ostage[:], rps[:])
                else:
                    nc.vector.tensor_copy(ostage[:], rps[:])
                nc.sync.dma_start(
                    out_e[:, rs * LANES : (rs + ROW) * LANES],
                    ostage[:],
                )

            def scan_body(_iv=None):
                emitted = set()
                n_ro = 0
                for r in range(STEPS):
                    emit_step(r)
                    if probe in ("noro", "noact"):
                        continue
                    rs = r - 5
                    if rs >= 0 and rs % ROW == 0:
                        emit_ro(rs, n_ro % 2 == 0)
                        emitted.add(rs)
                        n_ro += 1
                if probe not in ("noro", "noact"):
                    for rs in range(0, STEPS, ROW):
                        if rs not in emitted:
                            emit_ro(rs, n_ro % 2 == 0)
                            n_ro += 1

            if reps == 1:
                scan_body()
            else:
                with tc_ctx.For_i(0, reps, 1) as _i:
                    scan_body(_i)

    nc.compile()
    return nc


def host_prep(x, W_in, b_in, W_res, b_res, W_ro, t_total=T):
    """Produce the per-core in_maps (host-side layout/dtype prep only)."""
    a = np.float32(LEAKY)
    AT = (a * W_res).T.astype(np.float32)                     # [in, out]
    wres = (
        AT.reshape(NCH, 128, NCH, 128).transpose(1, 0, 2, 3).reshape(128, 16 * 128)
    ).astype(ml_dtypes.bfloat16)
    win = W_in.T.astype(ml_dtypes.bfloat16)                   # [128, 512]
    R = (a * W_ro).T.astype(np.float32)                       # [512, 128]
    wro = R.reshape(NCH, 128, 128).transpose(1, 0, 2).reshape(128, NCH * 128).astype(
        ml_dtypes.bfloat16
    )
    bias = (b_in + b_res).astype(np.float32).reshape(NCH, 128).T.copy()  # [128, 4]

    in_maps = []
    for c in range(NCORES):
        xt = np.empty((128, STEPS * LANES), np.float32)
        xv = xt.reshape(128, STEPS, 2, B)                      # [d, r, half, b]
        for half in range(2):
            t0 = slot_t0(2 * c + half)
            xv[:, :, half, :] = x[:, t0 : t0 + STEPS, :].transpose(2, 1, 0)
        in_maps.append({
            "xt": xt.astype(ml_dtypes.bfloat16),
            "wres": wres, "win": win, "wro": wro, "bias": bias,
        })
    return in_maps


_nc_cache = {}


def kernel(x, W_in, b_in, W_res, b_res, W_ro):
    """Full inputs in, full output out ([B, T, D] float32)."""
    global _last_results
    x, W_in, b_in, W_res, b_res, W_ro = (
        np.asarray(t, dtype=np.float32) for t in (x, W_in, b_in, W_res, b_res, W_ro)
    )
    t_total = x.shape[1]
    if t_total not in _nc_cache:
        _nc_cache[t_total] = build(t_total=t_total)
    nc = _nc_cache[t_total]

    in_maps = host_prep(x, W_in, b_in, W_res, b_res, W_ro, t_total=t_total)
    res = run_bass_kernel_spmd(nc, in_maps, list(range(NCORES)), trace=TRACE)
    _last_results = res

    out = np.empty((B, t_total, D), dtype=np.float32)
    for c in range(NCORES):
        oc = res.results[c]["out"].reshape(128, STEPS, 2, B)   # [d, r, half, b]
        for half in range(2):
            s = 2 * c + half
            t0 = slot_t0(s)
            u0 = 0 if s == 0 else WO
            out[:, t0 + u0 : t0 + u0 + SEGLEN, :] = (
                oc[:, u0 : u0 + SEGLEN, half, :].transpose(2, 1, 0)
            )
    return out


# revision 7
# speedup vs baseline: 1.4701x; 1.1534x over previous
"""AdaptiveESN Trainium2 kernel — dual fused-chain sequence-split (v4).

Echo State Network: B=64, T=2048, D=128, H=512, leaky a=0.26.
    h_t = (1-a) h_{t-1} + a tanh(x_t W_in^T + b_in + h_{t-1} W_res^T + b_res)
    y_t = h_t W_ro^T

The map is strongly contracting (state error decays ~0.74x/step), so a
chain restarted from h=0 converges to the true trajectory in ~32 steps.

Strategy: 32 overlapping slots of 96 steps (64 useful + 32 washout;
slot 0 starts at t=0 where h=0 is exact, so all its outputs are valid).
Core c runs TWO independent fused 128-lane chains (chain h = slots
4c+2h, 4c+2h+1), interleaved round-robin so one chain's matmuls hide the
other's cross-engine epilogue latency. Per chain-step: 24 matmuls of 128
cols (4 bias rank-1 outer products b_i (x) ones via K=1, 4 W_in, 16
W_res), ONE 512-wide tanh on ACT, ONE 512-wide fused AXPY on DVE
(h' = (1-a) h + p; state h~ = h/a with a folded into W_res/W_ro).
Readout (4 steps x 128 lanes per window) is spread one window per round.

Layouts (host-prepped, per core c; chain h covers slots s=4c+2h (lanes
0-63) and s+1 (lanes 64-127); t_s = 0 for s=0 else 64 s - 32):
    xt   bf16 [128, 2*96*128]  xt[d, (h*96+r)*128+half*64+b] = x[b, t_s+r, d]
    wres bf16 [128, 2048]      tile (j,i) at cols (j*4+i)*128: (a W_res).T block
    win  bf16 [128, 512]       W_in.T
    wro  bf16 [128, 512]       tile j at cols j*128: (a W_ro).T block
    biasr bf16 [1, 512]        b_in + b_res (rank-1 matmul stationary)
    out  f32  [128, 2*96*128]  out[d, (h*96+r)*128+half*64+b] = y[b, t_s+r, d]
Host keeps steps [0,64) of slot 0 and [32,96) of slots s>=1.
"""
import sys

if "/opt/trn_rl_repo" not in sys.path:
    sys.path.insert(0, "/opt/trn_rl_repo")

import numpy as np
import ml_dtypes

import concourse.bass as bass
from concourse import bacc
import concourse.mybir as mybir
import concourse.tile as tile
from concourse.bass_utils import run_bass_kernel_spmd

try:
    import jax

    jax.config.update("jax_compilation_cache_dir", "/tmp/jax_neff_cache")
    jax.config.update("jax_persistent_cache_min_compile_time_secs", 10)
except Exception:
    pass

B, T, D, H = 64, 2048, 128, 512
LEAKY = 0.26
NCORES = 8
NCH = H // 128            # H chunks (partition tiles)
SPC = 4                   # slots per core
NCHAIN = SPC // 2         # fused 128-lane chains per core
NSLOT = NCORES * SPC      # global slots
SEGLEN = T // NSLOT       # stride between slot starts (64)
WO = 32                   # discarded washout steps for slots >= 1
STEPS = SEGLEN + WO       # chain length (96)
LANES = 128               # lanes per fused chain (2 slots x 64 batch)
WST = NCH * LANES         # state cols per step (512)
TCB = 16                  # steps per state buffer
NBUF = 3                  # state buffers per chain
ROW = 4                   # steps per readout window (4*128 = 512 cols)
BF16 = mybir.dt.bfloat16
F32 = mybir.dt.float32

TRACE = False
_last_results = None


def slot_t0(s):
    return 0 if s == 0 else SEGLEN * s - WO


def build(t_total=T, tc=TCB, reps=1, probe=None, fat=True):
    """Build the per-core Bacc graph (same graph on all 8 cores).

    reps > 1 wraps the scan in a hardware For_i loop for wall-clock delta
    timing (per-scan = (wall_hi - wall_lo) / (reps_hi - reps_lo)).

    probe: timing-only structural variants (WRONG math, never for output):
      "zrhs"  - scan matmuls read h0 (zero) instead of hprev
      "noro"  - skip readout matmuls/copies/DMAs
      "nodve" - ACT writes states directly (no blend)
    fat=False: per-chunk ACT(+bias)/DVE epilogue (no bias matmuls).
    """
    assert t_total == T, "slot layout is hardcoded for T=2048"
    nc = bacc.Bacc(None, target_bir_lowering=False)
    xt_e = nc.declare_dram_parameter("xt", [128, NCHAIN * STEPS * LANES], BF16, isOutput=False)
    wres_e = nc.declare_dram_parameter("wres", [128, 16 * 128], BF16, isOutput=False)
    win_e = nc.declare_dram_parameter("win", [128, NCH * 128], BF16, isOutput=False)
    wro_e = nc.declare_dram_parameter("wro", [128, NCH * 128], BF16, isOutput=False)
    biasr_e = nc.declare_dram_parameter("biasr", [1, NCH * 128], BF16, isOutput=False)
    bias_e = nc.declare_dram_parameter("bias", [128, NCH], F32, isOutput=False)
    out_e = nc.declare_dram_parameter("out", [128, NCHAIN * STEPS * LANES], F32, isOutput=True)

    with tile.TileContext(nc) as tc_ctx:
        with (
            tc_ctx.tile_pool(name="const", bufs=1) as const_pool,
            tc_ctx.tile_pool(name="p", bufs=6) as p_pool,
            tc_ctx.tile_pool(name="ostage", bufs=3) as o_pool,
            tc_ctx.tile_pool(name="scan_ps", bufs=4, space=bass.MemorySpace.PSUM) as ps_pool,
            tc_ctx.tile_pool(name="ro_ps", bufs=2, space=bass.MemorySpace.PSUM) as ro_pool,
        ):
            xt_sb = const_pool.tile([128, NCHAIN * STEPS * LANES], BF16)
            wres_sb = const_pool.tile([128, 16 * 128], BF16)
            win_sb = const_pool.tile([128, NCH * 128], BF16)
            wro_sb = const_pool.tile([128, NCH * 128], BF16)
            biasr_sb = const_pool.tile([1, NCH * 128], BF16)
            bias_sb = const_pool.tile([128, NCH], F32)
            ones_sb = const_pool.tile([1, LANES], BF16)
            h0_sb = const_pool.tile([128, WST], BF16)
            # states per chain, step-major: col (r%TCB)*WST + i*LANES + lane
            st = [
                [
                    const_pool.tile([128, TCB * WST], BF16, name=f"st{h}_{n}", tag=f"st{h}_{n}")
                    for n in range(NBUF)
                ]
                for h in range(NCHAIN)
            ]

            nc.sync.dma_start(wres_sb[:], wres_e[:])
            nc.sync.dma_start(win_sb[:], win_e[:])
            nc.sync.dma_start(wro_sb[:], wro_e[:])
            nc.sync.dma_start(biasr_sb[:], biasr_e[:])
            nc.sync.dma_start(bias_sb[:], bias_e[:])
            nc.sync.dma_start(xt_sb[:], xt_e[:])
            nc.vector.memset(ones_sb[:], 1.0)
            nc.vector.memset(h0_sb[:], 0.0)

            def emit_step(h, r):
                if r == 0:
                    hprev = h0_sb[:]
                else:
                    bprev = ((r - 1) // TCB) % NBUF
                    sprev = (r - 1) % TCB
                    hprev = st[h][bprev][:, sprev * WST : (sprev + 1) * WST]
                bcur = (r // TCB) % NBUF
                scur = r % TCB
                xcol = xt_sb[:, (h * STEPS + r) * LANES : (h * STEPS + r + 1) * LANES]
                hsrc = h0_sb[:] if probe == "zrhs" else hprev

                ps = ps_pool.tile([128, WST], F32)

                def psw(i):
                    return ps[:, i * LANES : (i + 1) * LANES]

                def hcol(j):
                    return hsrc[:, j * LANES : (j + 1) * LANES]

                # per-region accumulation groups stay sequential (interleaved
                # groups on one PSUM tile mis-accumulate); (bias, win) first
                # have no state dependency, j=3 deferred last.
                for i in range(NCH):
                    ops = [(win_sb[:, i * 128 : (i + 1) * 128], xcol)]
                    if fat:
                        ops.insert(0, (biasr_sb[:, i * 128 : (i + 1) * 128], ones_sb[:]))
                    ops += [
                        (wres_sb[:, (j * NCH + i) * 128 : (j * NCH + i + 1) * 128], hcol(j))
                        for j in range(NCH)
                    ]
                    for kk, (lhsT, rhs) in enumerate(ops):
                        nc.tensor.matmul(
                            psw(i), lhsT, rhs,
                            start=(kk == 0), stop=(kk == len(ops) - 1))

                st_step = st[h][bcur][:, scur * WST : (scur + 1) * WST]
                if fat:
                    if probe == "nodve":
                        nc.scalar.activation(
                            st_step, ps[:], mybir.ActivationFunctionType.Tanh)
                    else:
                        p_t = p_pool.tile([128, WST], BF16)
                        nc.scalar.activation(
                            p_t[:], ps[:], mybir.ActivationFunctionType.Tanh)
                        nc.vector.scalar_tensor_tensor(
                            st_step, hprev, 1.0 - LEAKY, p_t[:],
                            op0=mybir.AluOpType.mult, op1=mybir.AluOpType.add)
                else:
                    for i in range(NCH):
                        st_col = st_step[:, i * LANES : (i + 1) * LANES]
                        bias_ap = bias_sb[:, i : i + 1]
                        if probe == "nodve":
                            nc.scalar.activation(
                                st_col, psw(i), mybir.ActivationFunctionType.Tanh,
                                bias=bias_ap)
                        else:
                            p_t = p_pool.tile([128, LANES], BF16)
                            nc.scalar.activation(
                                p_t[:], psw(i), mybir.ActivationFunctionType.Tanh,
                                bias=bias_ap)
                            nc.vector.scalar_tensor_tensor(
                                st_col,
                                hprev[:, i * LANES : (i + 1) * LANES],
                                1.0 - LEAKY, p_t[:],
                                op0=mybir.AluOpType.mult, op1=mybir.AluOpType.add)

            def emit_ro(h, rs, alt):
                # readout of chain h states for steps [rs, rs+ROW), 128 lanes
                b = (rs // TCB) % NBUF
                ls = rs % TCB
                st_v = st[h][b].rearrange("p (s w) -> p s w", w=WST)
                rps = ro_pool.tile([128, ROW * LANES], F32)
                for j in range(NCH):
                    nc.tensor.matmul(
                        rps[:],
                        wro_sb[:, j * 128 : (j + 1) * 128],
                        st_v[:, ls : ls + ROW, j * LANES : (j + 1) * LANES],
                        start=(j == 0),
                        stop=(j == NCH - 1),
                    )
                ostage = o_pool.tile([128, ROW * LANES], F32)
                if alt:
                    nc.scalar.copy(ostage[:], rps[:])
                else:
                    nc.vector.tensor_copy(ostage[:], rps[:])
                nc.sync.dma_start(
                    out_e[:, (h * STEPS + rs) * LANES : (h * STEPS + rs + ROW) * LANES],
                    ostage[:],
                )

            def scan_body(_iv=None):
                # windows in production order; one emitted per round
                windows = [
                    (h, rs)
                    for rs in range(0, STEPS, ROW)
                    for h in range(NCHAIN)
                ]
                n_ro = 0
                for r in range(STEPS):
                    for h in range(NCHAIN):
                        emit_step(h, r)
                    if probe == "noro":
                        continue
                    if n_ro < len(windows):
                        h, rs = windows[n_ro]
                        if rs + ROW <= r:  # steps of the window are done
                            emit_ro(h, rs, n_ro % 2 == 0)
                            n_ro += 1
                if probe != "noro":
                    while n_ro < len(windows):
                        h, rs = windows[n_ro]
                        emit_ro(h, rs, n_ro % 2 == 0)
                        n_ro += 1

            if reps == 1:
                scan_body()
            else:
                with tc_ctx.For_i(0, reps, 1) as _i:
                    scan_body(_i)

    nc.compile()
    return nc


def host_prep(x, W_in, b_in, W_res, b_res, W_ro, t_total=T):
    """Produce the per-core in_maps (host-side layout/dtype prep only)."""
    a = np.float32(LEAKY)
    AT = (a * W_res).T.astype(np.float32)                     # [in, out]
    wres = (
        AT.reshape(NCH, 128, NCH, 128).transpose(1, 0, 2, 3).reshape(128, 16 * 128)
    ).astype(ml_dtypes.bfloat16)
    win = W_in.T.astype(ml_dtypes.bfloat16)                   # [128, 512]
    R = (a * W_ro).T.astype(np.float32)                       # [512, 128]
    wro = R.reshape(NCH, 128, 128).transpose(1, 0, 2).reshape(128, NCH * 128).astype(
        ml_dtypes.bfloat16
    )
    bvec = (b_in + b_res).astype(np.float32)
    biasr = bvec.reshape(1, NCH * 128).astype(ml_dtypes.bfloat16)
    bias = bvec.reshape(NCH, 128).T.copy()                    # [128, 4]

    in_maps = []
    for c in range(NCORES):
        xt = np.empty((128, NCHAIN * STEPS * LANES), np.float32)
        xv = xt.reshape(128, NCHAIN, STEPS, 2, B)             # [d, h, r, half, b]
        for h in range(NCHAIN):
            for half in range(2):
                t0 = slot_t0(SPC * c + 2 * h + half)
                xv[:, h, :, half, :] = x[:, t0 : t0 + STEPS, :].transpose(2, 1, 0)
        in_maps.append({
            "xt": xt.astype(ml_dtypes.bfloat16),
            "wres": wres, "win": win, "wro": wro,
            "biasr": biasr, "bias": bias,
        })
    return in_maps


_nc_cache = {}


def kernel(x, W_in, b_in, W_res, b_res, W_ro):
    """Full inputs in, full output out ([B, T, D] float32)."""
    global _last_results
    x, W_in, b_in, W_res, b_res, W_ro = (
        np.asarray(t, dtype=np.float32) for t in (x, W_in, b_in, W_res, b_res, W_ro)
    )
    t_total = x.shape[1]
    if t_total not in _nc_cache:
        _nc_cache[t_total] = build(t_total=t_total)
    nc = _nc_cache[t_total]

    in_maps = host_prep(x, W_in, b_in, W_res, b_res, W_ro, t_total=t_total)
    res = run_bass_kernel_spmd(nc, in_maps, list(range(NCORES)), trace=TRACE)
    _last_results = res

    out = np.empty((B, t_total, D), dtype=np.float32)
    for c in range(NCORES):
        oc = res.results[c]["out"].reshape(128, NCHAIN, STEPS, 2, B)
        for h in range(NCHAIN):
            for half in range(2):
                s = SPC * c + 2 * h + half
                t0 = slot_t0(s)
                u0 = 0 if s == 0 else WO
                out[:, t0 + u0 : t0 + u0 + SEGLEN, :] = (
                    oc[:, h, u0 : u0 + SEGLEN, half, :].transpose(2, 1, 0)
                )
    return out


# revision 10
# speedup vs baseline: 2.7153x; 1.8470x over previous
"""AdaptiveESN Trainium2 kernel — dual fused-chain sequence-split (v4).

Echo State Network: B=64, T=2048, D=128, H=512, leaky a=0.26.
    h_t = (1-a) h_{t-1} + a tanh(x_t W_in^T + b_in + h_{t-1} W_res^T + b_res)
    y_t = h_t W_ro^T

The map is strongly contracting (state error decays ~0.74x/step), so a
chain restarted from h=0 converges to the true trajectory in ~32 steps.

Strategy: 32 overlapping slots of 96 steps (64 useful + 32 washout;
slot 0 starts at t=0 where h=0 is exact, so all its outputs are valid).
Core c runs TWO independent fused 128-lane chains (chain h = slots
4c+2h, 4c+2h+1), interleaved round-robin so one chain's matmuls hide the
other's cross-engine epilogue latency. Per chain-step: 24 matmuls of 128
cols (4 bias rank-1 outer products b_i (x) ones via K=1, 4 W_in, 16
W_res), ONE 512-wide tanh on ACT, ONE 512-wide fused AXPY on DVE
(h' = (1-a) h + p; state h~ = h/a with a folded into W_res/W_ro).
Readout (4 steps x 128 lanes per window) is spread one window per round.

Layouts (host-prepped, per core c; chain h covers slots s=4c+2h (lanes
0-63) and s+1 (lanes 64-127); t_s = 0 for s=0 else 64 s - 32):
    xt   bf16 [128, 2*96*128]  xt[d, (h*96+r)*128+half*64+b] = x[b, t_s+r, d]
    wres bf16 [128, 2048]      tile (j,i) at cols (j*4+i)*128: (a W_res).T block
    win  bf16 [128, 512]       W_in.T
    wro  bf16 [128, 512]       tile j at cols j*128: (a W_ro).T block
    biasr bf16 [1, 512]        b_in + b_res (rank-1 matmul stationary)
    out  f32  [128, 2*96*128]  out[d, (h*96+r)*128+half*64+b] = y[b, t_s+r, d]
Host keeps steps [0,64) of slot 0 and [32,96) of slots s>=1.
"""
import sys

if "/opt/trn_rl_repo" not in sys.path:
    sys.path.insert(0, "/opt/trn_rl_repo")

import numpy as np
import ml_dtypes

import concourse.bass as bass
from concourse import bacc
import concourse.mybir as mybir
import concourse.tile as tile
from concourse.bass_utils import run_bass_kernel_spmd

try:
    import jax

    jax.config.update("jax_compilation_cache_dir", "/tmp/jax_neff_cache")
    jax.config.update("jax_persistent_cache_min_compile_time_secs", 10)
except Exception:
    pass

B, T, D, H = 64, 2048, 128, 512
LEAKY = 0.26
NCORES = 8
NCH = H // 128            # H chunks (partition tiles)
SPC = 4                   # slots per core
NCHAIN = SPC // 2         # fused 128-lane chains per core
NSLOT = NCORES * SPC      # global slots
SEGLEN = T // NSLOT       # stride between slot starts (64)
WO = 24                   # discarded washout steps for slots >= 1
STEPS = SEGLEN + WO       # chain length (96)
LANES = 128               # lanes per fused chain (2 slots x 64 batch)
WST = NCH * LANES         # state cols per step (512)
TCB = 8                   # steps per state buffer
NBUF = 3                  # state buffers per chain
ROW = 4                   # steps per readout window (4*128 = 512 cols)
BF16 = mybir.dt.bfloat16
F32 = mybir.dt.float32

TRACE = False
_last_results = None


def slot_t0(s):
    return 0 if s == 0 else SEGLEN * s - WO


def build(t_total=T, tc=TCB, reps=1, probe=None, fat=False, fatdve=False, rodma=False):
    """Build the per-core Bacc graph (same graph on all 8 cores).

    reps > 1 wraps the scan in a hardware For_i loop for wall-clock delta
    timing (per-scan = (wall_hi - wall_lo) / (reps_hi - reps_lo)).

    probe: timing-only structural variants (WRONG math, never for output):
      "zrhs"  - scan matmuls read h0 (zero) instead of hprev
      "noro"  - skip readout matmuls/copies/DMAs
      "nodve" - ACT writes states directly (no blend)
    fat=False: per-chunk ACT(+bias)/DVE epilogue (no bias matmuls).
    """
    assert t_total == T, "slot layout is hardcoded for T=2048"
    nc = bacc.Bacc(None, target_bir_lowering=False)
    xt_e = nc.declare_dram_parameter("xt", [128, NCHAIN * STEPS * LANES], BF16, isOutput=False)
    wres_e = nc.declare_dram_parameter("wres", [128, 16 * 128], BF16, isOutput=False)
    win_e = nc.declare_dram_parameter("win", [128, NCH * 128], BF16, isOutput=False)
    wro_e = nc.declare_dram_parameter("wro", [128, NCH * 128], BF16, isOutput=False)
    biasr_e = nc.declare_dram_parameter("biasr", [1, NCH * 128], BF16, isOutput=False)
    bias_e = nc.declare_dram_parameter("bias", [128, NCH], F32, isOutput=False)
    out_e = nc.declare_dram_parameter("out", [128, NCHAIN * STEPS * LANES], F32, isOutput=True)

    with tile.TileContext(nc) as tc_ctx:
        with (
            tc_ctx.tile_pool(name="const", bufs=1) as const_pool,
            tc_ctx.tile_pool(name="p", bufs=6) as p_pool,
            tc_ctx.tile_pool(name="ostage", bufs=3) as o_pool,
            tc_ctx.tile_pool(name="scan_ps", bufs=4, space=bass.MemorySpace.PSUM) as ps_pool,
            tc_ctx.tile_pool(name="ro_ps", bufs=2, space=bass.MemorySpace.PSUM) as ro_pool,
        ):
            xt_sb = const_pool.tile([128, NCHAIN * STEPS * LANES], BF16)
            wres_sb = const_pool.tile([128, 16 * 128], BF16)
            win_sb = const_pool.tile([128, NCH * 128], BF16)
            wro_sb = const_pool.tile([128, NCH * 128], BF16)
            biasr_sb = const_pool.tile([1, NCH * 128], BF16)
            bias_sb = const_pool.tile([128, NCH], F32)
            ones_sb = const_pool.tile([1, LANES], BF16)
            h0_sb = const_pool.tile([128, WST], BF16)
            # states per chain, step-major: col (r%TCB)*WST + i*LANES + lane
            st = [
                [
                    const_pool.tile([128, TCB * WST], BF16, name=f"st{h}_{n}", tag=f"st{h}_{n}")
                    for n in range(NBUF)
                ]
                for h in range(NCHAIN)
            ]

            nc.sync.dma_start(wres_sb[:], wres_e[:])
            nc.sync.dma_start(win_sb[:], win_e[:])
            nc.sync.dma_start(wro_sb[:], wro_e[:])
            nc.sync.dma_start(biasr_sb[:], biasr_e[:])
            nc.sync.dma_start(bias_sb[:], bias_e[:])
            nc.sync.dma_start(xt_sb[:], xt_e[:])
            nc.vector.memset(ones_sb[:], 1.0)
            nc.vector.memset(h0_sb[:], 0.0)

            def emit_step(h, r):
                if r == 0:
                    hprev = h0_sb[:]
                else:
                    bprev = ((r - 1) // TCB) % NBUF
                    sprev = (r - 1) % TCB
                    hprev = st[h][bprev][:, sprev * WST : (sprev + 1) * WST]
                bcur = (r // TCB) % NBUF
                scur = r % TCB
                xcol = xt_sb[:, (h * STEPS + r) * LANES : (h * STEPS + r + 1) * LANES]
                hsrc = h0_sb[:] if probe == "zrhs" else hprev

                ps = ps_pool.tile([128, WST], F32)

                def psw(i):
                    return ps[:, i * LANES : (i + 1) * LANES]

                def hcol(j):
                    return hsrc[:, j * LANES : (j + 1) * LANES]

                # per-region accumulation groups stay sequential (interleaved
                # groups on one PSUM tile mis-accumulate); (bias, win) first
                # have no state dependency, j=3 deferred last.
                for i in range(NCH):
                    ops = [(win_sb[:, i * 128 : (i + 1) * 128], xcol)]
                    if fat:
                        ops.insert(0, (biasr_sb[:, i * 128 : (i + 1) * 128], ones_sb[:]))
                    ops += [
                        (wres_sb[:, (j * NCH + i) * 128 : (j * NCH + i + 1) * 128], hcol(j))
                        for j in range(NCH)
                    ]
                    for kk, (lhsT, rhs) in enumerate(ops):
                        nc.tensor.matmul(
                            psw(i), lhsT, rhs,
                            start=(kk == 0), stop=(kk == len(ops) - 1))

                st_step = st[h][bcur][:, scur * WST : (scur + 1) * WST]
                if fat:
                    if probe == "nodve":
                        nc.scalar.activation(
                            st_step, ps[:], mybir.ActivationFunctionType.Tanh)
                    else:
                        p_t = p_pool.tile([128, WST], BF16)
                        nc.scalar.activation(
                            p_t[:], ps[:], mybir.ActivationFunctionType.Tanh)
                        nc.vector.scalar_tensor_tensor(
                            st_step, hprev, 1.0 - LEAKY, p_t[:],
                            op0=mybir.AluOpType.mult, op1=mybir.AluOpType.add)
                elif fatdve:
                    # 4 thin tanh (per-chunk bias) into one p tile, 1 AXPY
                    p_t = p_pool.tile([128, WST], BF16)
                    for i in range(NCH):
                        nc.scalar.activation(
                            p_t[:, i * LANES : (i + 1) * LANES], psw(i),
                            mybir.ActivationFunctionType.Tanh,
                            bias=bias_sb[:, i : i + 1])
                    nc.vector.scalar_tensor_tensor(
                        st_step, hprev, 1.0 - LEAKY, p_t[:],
                        op0=mybir.AluOpType.mult, op1=mybir.AluOpType.add)
                else:
                    for i in range(NCH):
                        st_col = st_step[:, i * LANES : (i + 1) * LANES]
                        bias_ap = bias_sb[:, i : i + 1]
                        if probe == "nodve":
                            nc.scalar.activation(
                                st_col, psw(i), mybir.ActivationFunctionType.Tanh,
                                bias=bias_ap)
                        else:
                            p_t = p_pool.tile([128, LANES], BF16)
                            nc.scalar.activation(
                                p_t[:], psw(i), mybir.ActivationFunctionType.Tanh,
                                bias=bias_ap)
                            nc.vector.scalar_tensor_tensor(
                                st_col,
                                hprev[:, i * LANES : (i + 1) * LANES],
                                1.0 - LEAKY, p_t[:],
                                op0=mybir.AluOpType.mult, op1=mybir.AluOpType.add)

            def emit_ro(h, rs, alt):
                # readout of chain h states for steps [rs, rs+ROW), 128 lanes
                b = (rs // TCB) % NBUF
                ls = rs % TCB
                st_v = st[h][b].rearrange("p (s w) -> p s w", w=WST)
                rps = ro_pool.tile([128, ROW * LANES], F32)
                for j in range(NCH):
                    nc.tensor.matmul(
                        rps[:],
                        wro_sb[:, j * 128 : (j + 1) * 128],
                        st_v[:, ls : ls + ROW, j * LANES : (j + 1) * LANES],
                        start=(j == 0),
                        stop=(j == NCH - 1),
                    )
                dst = out_e[:, (h * STEPS + rs) * LANES : (h * STEPS + rs + ROW) * LANES]
                if rodma:
                    nc.sync.dma_start(dst, rps[:])
                else:
                    ostage = o_pool.tile([128, ROW * LANES], F32)
                    if alt:
                        nc.scalar.copy(ostage[:], rps[:])
                    else:
                        nc.vector.tensor_copy(ostage[:], rps[:])
                    nc.sync.dma_start(dst, ostage[:])

            def scan_body(_iv=None):
                # windows in production order; one emitted per round
                windows = [
                    (h, rs)
                    for rs in range(0, STEPS, ROW)
                    for h in range(NCHAIN)
                ]
                n_ro = 0
                for r in range(STEPS):
                    for h in range(NCHAIN):
                        emit_step(h, r)
                    if probe == "noro":
                        continue
                    if n_ro < len(windows):
                        h, rs = windows[n_ro]
                        if rs + ROW <= r:  # steps of the window are done
                            emit_ro(h, rs, n_ro % 2 == 0)
                            n_ro += 1
                if probe != "noro":
                    while n_ro < len(windows):
                        h, rs = windows[n_ro]
                        emit_ro(h, rs, n_ro % 2 == 0)
                        n_ro += 1

            if reps == 1:
                scan_body()
            else:
                with tc_ctx.For_i(0, reps, 1) as _i:
                    scan_body(_i)

    nc.compile()
    return nc


def host_prep(x, W_in, b_in, W_res, b_res, W_ro, t_total=T):
    """Produce the per-core in_maps (host-side layout/dtype prep only)."""
    a = np.float32(LEAKY)
    AT = (a * W_res).T.astype(np.float32)                     # [in, out]
    wres = (
        AT.reshape(NCH, 128, NCH, 128).transpose(1, 0, 2, 3).reshape(128, 16 * 128)
    ).astype(ml_dtypes.bfloat16)
    win = W_in.T.astype(ml_dtypes.bfloat16)                   # [128, 512]
    R = (a * W_ro).T.astype(np.float32)                       # [512, 128]
    wro = R.reshape(NCH, 128, 128).transpose(1, 0, 2).reshape(128, NCH * 128).astype(
        ml_dtypes.bfloat16
    )
    bvec = (b_in + b_res).astype(np.float32)
    biasr = bvec.reshape(1, NCH * 128).astype(ml_dtypes.bfloat16)
    bias = bvec.reshape(NCH, 128).T.copy()                    # [128, 4]

    in_maps = []
    for c in range(NCORES):
        xt = np.empty((128, NCHAIN * STEPS * LANES), np.float32)
        xv = xt.reshape(128, NCHAIN, STEPS, 2, B)             # [d, h, r, half, b]
        for h in range(NCHAIN):
            for half in range(2):
                t0 = slot_t0(SPC * c + 2 * h + half)
                xv[:, h, :, half, :] = x[:, t0 : t0 + STEPS, :].transpose(2, 1, 0)
        in_maps.append({
            "xt": xt.astype(ml_dtypes.bfloat16),
            "wres": wres, "win": win, "wro": wro,
            "biasr": biasr, "bias": bias,
        })
    return in_maps


_nc_cache = {}


def kernel(x, W_in, b_in, W_res, b_res, W_ro):
    """Full inputs in, full output out ([B, T, D] float32)."""
    global _last_results
    x, W_in, b_in, W_res, b_res, W_ro = (
        np.asarray(t, dtype=np.float32) for t in (x, W_in, b_in, W_res, b_res, W_ro)
    )
    t_total = x.shape[1]
    if t_total not in _nc_cache:
        _nc_cache[t_total] = build(t_total=t_total)
    nc = _nc_cache[t_total]

    in_maps = host_prep(x, W_in, b_in, W_res, b_res, W_ro, t_total=t_total)
    res = run_bass_kernel_spmd(nc, in_maps, list(range(NCORES)), trace=TRACE)
    _last_results = res

    out = np.empty((B, t_total, D), dtype=np.float32)
    for c in range(NCORES):
        oc = res.results[c]["out"].reshape(128, NCHAIN, STEPS, 2, B)
        for h in range(NCHAIN):
            for half in range(2):
                s = SPC * c + 2 * h + half
                t0 = slot_t0(s)
                u0 = 0 if s == 0 else WO
                out[:, t0 + u0 : t0 + u0 + SEGLEN, :] = (
                    oc[:, h, u0 : u0 + SEGLEN, half, :].transpose(2, 1, 0)
                )
    return out


# revision 11
# speedup vs baseline: 2.7799x; 1.0238x over previous
"""AdaptiveESN Trainium2 kernel — dual fused-chain sequence-split (v4).

Echo State Network: B=64, T=2048, D=128, H=512, leaky a=0.26.
    h_t = (1-a) h_{t-1} + a tanh(x_t W_in^T + b_in + h_{t-1} W_res^T + b_res)
    y_t = h_t W_ro^T

The map is strongly contracting (state error decays ~0.74x/step), so a
chain restarted from h=0 converges to the true trajectory in ~32 steps.

Strategy: 32 overlapping slots of 88 steps (64 useful + 24 washout;
slot 0 starts at t=0 where h=0 is exact, so all its outputs are valid).
Core c runs TWO independent fused 128-lane chains (chain h = slots
4c+2h, 4c+2h+1), interleaved round-robin so one chain's matmuls hide the
other's cross-engine (PE->ACT->DVE->PE) epilogue latency. Per chain-step:
20 matmuls of 128 cols (16 W_res tiles as stationary + 4 W_in), with the
j=3 contraction chunk deferred last in each accumulation group; then 4
per-chunk tanh on ACT (bias via per-partition ACT bias) and 4 fused AXPY
blends on DVE (h' = (1-a) h + p; state h~ = h/a with a folded into
W_res/W_ro so the blend is one scalar_tensor_tensor). Readout (4 steps x
128 lanes per window) is spread ~one window per round between scan steps.
The dominant cost on this part is ~50 ns of sync/dispatch overhead per
instruction, so everything is shaped to minimize instruction count at
maximum tile width; PSUM accumulation groups must stay sequential per
region (interleaving groups on one PSUM tile mis-accumulates).

Layouts (host-prepped, per core c; chain h covers slots s=4c+2h (lanes
0-63) and s+1 (lanes 64-127); t_s = 0 for s=0 else 64 s - 24):
    xt   bf16 [128, 2*88*128]  xt[d, (h*88+r)*128+half*64+b] = x[b, t_s+r, d]
    wres bf16 [128, 2048]      tile (j,i) at cols (j*4+i)*128: (a W_res).T block
    win  bf16 [128, 512]       W_in.T
    wro  bf16 [128, 512]       tile j at cols j*128: (a W_ro).T block
    bias f32  [128, 4]         (b_in + b_res) chunk i in col i
    out  f32  [128, 2*88*128]  out[d, (h*88+r)*128+half*64+b] = y[b, t_s+r, d]
Host keeps steps [0,64) of slot 0 and [24,88) of slots s>=1.
"""
import sys

if "/opt/trn_rl_repo" not in sys.path:
    sys.path.insert(0, "/opt/trn_rl_repo")

import numpy as np
import ml_dtypes

import concourse.bass as bass
from concourse import bacc
import concourse.mybir as mybir
import concourse.tile as tile
from concourse.bass_utils import run_bass_kernel_spmd

try:
    import jax

    jax.config.update("jax_compilation_cache_dir", "/tmp/jax_neff_cache")
    jax.config.update("jax_persistent_cache_min_compile_time_secs", 10)
except Exception:
    pass

B, T, D, H = 64, 2048, 128, 512
LEAKY = 0.26
NCORES = 8
NCH = H // 128            # H chunks (partition tiles)
SPC = 4                   # slots per core
NCHAIN = SPC // 2         # fused 128-lane chains per core
NSLOT = NCORES * SPC      # global slots
SEGLEN = T // NSLOT       # stride between slot starts (64)
WO = 24                   # discarded washout steps for slots >= 1
STEPS = SEGLEN + WO       # chain length (96)
LANES = 128               # lanes per fused chain (2 slots x 64 batch)
WST = NCH * LANES         # state cols per step (512)
TCB = 8                   # steps per state buffer
NBUF = 3                  # state buffers per chain
ROW = 4                   # steps per readout window (4*128 = 512 cols)
BF16 = mybir.dt.bfloat16
F32 = mybir.dt.float32

TRACE = False
_last_results = None


def slot_t0(s):
    return 0 if s == 0 else SEGLEN * s - WO


def build(t_total=T, tc=TCB, reps=1, probe=None, fat=False, fatdve=False, rodma=False):
    """Build the per-core Bacc graph (same graph on all 8 cores).

    reps > 1 wraps the scan in a hardware For_i loop for wall-clock delta
    timing (per-scan = (wall_hi - wall_lo) / (reps_hi - reps_lo)).

    probe: timing-only structural variants (WRONG math, never for output):
      "zrhs"  - scan matmuls read h0 (zero) instead of hprev
      "noro"  - skip readout matmuls/copies/DMAs
      "nodve" - ACT writes states directly (no blend)
    fat=False: per-chunk ACT(+bias)/DVE epilogue (no bias matmuls).
    """
    assert t_total == T, "slot layout is hardcoded for T=2048"
    nc = bacc.Bacc(None, target_bir_lowering=False)
    xt_e = nc.declare_dram_parameter("xt", [128, NCHAIN * STEPS * LANES], BF16, isOutput=False)
    wres_e = nc.declare_dram_parameter("wres", [128, 16 * 128], BF16, isOutput=False)
    win_e = nc.declare_dram_parameter("win", [128, NCH * 128], BF16, isOutput=False)
    wro_e = nc.declare_dram_parameter("wro", [128, NCH * 128], BF16, isOutput=False)
    biasr_e = nc.declare_dram_parameter("biasr", [1, NCH * 128], BF16, isOutput=False)
    bias_e = nc.declare_dram_parameter("bias", [128, NCH], F32, isOutput=False)
    out_e = nc.declare_dram_parameter("out", [128, NCHAIN * STEPS * LANES], F32, isOutput=True)

    with tile.TileContext(nc) as tc_ctx:
        with (
            tc_ctx.tile_pool(name="const", bufs=1) as const_pool,
            tc_ctx.tile_pool(name="p", bufs=6) as p_pool,
            tc_ctx.tile_pool(name="ostage", bufs=3) as o_pool,
            tc_ctx.tile_pool(name="scan_ps", bufs=4, space=bass.MemorySpace.PSUM) as ps_pool,
            tc_ctx.tile_pool(name="ro_ps", bufs=2, space=bass.MemorySpace.PSUM) as ro_pool,
        ):
            xt_sb = const_pool.tile([128, NCHAIN * STEPS * LANES], BF16)
            wres_sb = const_pool.tile([128, 16 * 128], BF16)
            win_sb = const_pool.tile([128, NCH * 128], BF16)
            wro_sb = const_pool.tile([128, NCH * 128], BF16)
            biasr_sb = const_pool.tile([1, NCH * 128], BF16)
            bias_sb = const_pool.tile([128, NCH], F32)
            ones_sb = const_pool.tile([1, LANES], BF16)
            h0_sb = const_pool.tile([128, WST], BF16)
            # states per chain, step-major: col (r%TCB)*WST + i*LANES + lane
            st = [
                [
                    const_pool.tile([128, TCB * WST], BF16, name=f"st{h}_{n}", tag=f"st{h}_{n}")
                    for n in range(NBUF)
                ]
                for h in range(NCHAIN)
            ]

            nc.sync.dma_start(wres_sb[:], wres_e[:])
            nc.sync.dma_start(win_sb[:], win_e[:])
            nc.sync.dma_start(wro_sb[:], wro_e[:])
            nc.sync.dma_start(biasr_sb[:], biasr_e[:])
            nc.sync.dma_start(bias_sb[:], bias_e[:])
            nc.sync.dma_start(xt_sb[:], xt_e[:])
            nc.vector.memset(ones_sb[:], 1.0)
            nc.vector.memset(h0_sb[:], 0.0)

            def emit_step(h, r):
                if r == 0:
                    hprev = h0_sb[:]
                else:
                    bprev = ((r - 1) // TCB) % NBUF
                    sprev = (r - 1) % TCB
                    hprev = st[h][bprev][:, sprev * WST : (sprev + 1) * WST]
                bcur = (r // TCB) % NBUF
                scur = r % TCB
                xcol = xt_sb[:, (h * STEPS + r) * LANES : (h * STEPS + r + 1) * LANES]
                hsrc = h0_sb[:] if probe == "zrhs" else hprev

                ps = ps_pool.tile([128, WST], F32)

                def psw(i):
                    return ps[:, i * LANES : (i + 1) * LANES]

                def hcol(j):
                    return hsrc[:, j * LANES : (j + 1) * LANES]

                # per-region accumulation groups stay sequential (interleaved
                # groups on one PSUM tile mis-accumulate); (bias, win) first
                # have no state dependency, j=3 deferred last.
                for i in range(NCH):
                    ops = [(win_sb[:, i * 128 : (i + 1) * 128], xcol)]
                    if fat:
                        ops.insert(0, (biasr_sb[:, i * 128 : (i + 1) * 128], ones_sb[:]))
                    ops += [
                        (wres_sb[:, (j * NCH + i) * 128 : (j * NCH + i + 1) * 128], hcol(j))
                        for j in range(NCH)
                    ]
                    for kk, (lhsT, rhs) in enumerate(ops):
                        nc.tensor.matmul(
                            psw(i), lhsT, rhs,
                            start=(kk == 0), stop=(kk == len(ops) - 1))

                st_step = st[h][bcur][:, scur * WST : (scur + 1) * WST]
                if fat:
                    if probe == "nodve":
                        nc.scalar.activation(
                            st_step, ps[:], mybir.ActivationFunctionType.Tanh)
                    else:
                        p_t = p_pool.tile([128, WST], BF16)
                        nc.scalar.activation(
                            p_t[:], ps[:], mybir.ActivationFunctionType.Tanh)
                        nc.vector.scalar_tensor_tensor(
                            st_step, hprev, 1.0 - LEAKY, p_t[:],
                            op0=mybir.AluOpType.mult, op1=mybir.AluOpType.add)
                elif fatdve:
                    # 4 thin tanh (per-chunk bias) into one p tile, 1 AXPY
                    p_t = p_pool.tile([128, WST], BF16)
                    for i in range(NCH):
                        nc.scalar.activation(
                            p_t[:, i * LANES : (i + 1) * LANES], psw(i),
                            mybir.ActivationFunctionType.Tanh,
                            bias=bias_sb[:, i : i + 1])
                    nc.vector.scalar_tensor_tensor(
                        st_step, hprev, 1.0 - LEAKY, p_t[:],
                        op0=mybir.AluOpType.mult, op1=mybir.AluOpType.add)
                else:
                    for i in range(NCH):
                        st_col = st_step[:, i * LANES : (i + 1) * LANES]
                        bias_ap = bias_sb[:, i : i + 1]
                        if probe == "nodve":
                            nc.scalar.activation(
                                st_col, psw(i), mybir.ActivationFunctionType.Tanh,
                                bias=bias_ap)
                        else:
                            p_t = p_pool.tile([128, LANES], BF16)
                            nc.scalar.activation(
                                p_t[:], psw(i), mybir.ActivationFunctionType.Tanh,
                                bias=bias_ap)
                            nc.vector.scalar_tensor_tensor(
                                st_col,
                                hprev[:, i * LANES : (i + 1) * LANES],
                                1.0 - LEAKY, p_t[:],
                                op0=mybir.AluOpType.mult, op1=mybir.AluOpType.add)

            def emit_ro(h, rs, alt):
                # readout of chain h states for steps [rs, rs+ROW), 128 lanes
                b = (rs // TCB) % NBUF
                ls = rs % TCB
                st_v = st[h][b].rearrange("p (s w) -> p s w", w=WST)
                rps = ro_pool.tile([128, ROW * LANES], F32)
                for j in range(NCH):
                    nc.tensor.matmul(
                        rps[:],
                        wro_sb[:, j * 128 : (j + 1) * 128],
                        st_v[:, ls : ls + ROW, j * LANES : (j + 1) * LANES],
                        start=(j == 0),
                        stop=(j == NCH - 1),
                    )
                dst = out_e[:, (h * STEPS + rs) * LANES : (h * STEPS + rs + ROW) * LANES]
                if rodma:
                    nc.sync.dma_start(dst, rps[:])
                else:
                    ostage = o_pool.tile([128, ROW * LANES], F32)
                    if alt:
                        nc.scalar.copy(ostage[:], rps[:])
                    else:
                        nc.vector.tensor_copy(ostage[:], rps[:])
                    nc.sync.dma_start(dst, ostage[:])

            def scan_body(_iv=None):
                # windows in production order; one emitted per round
                windows = [
                    (h, rs)
                    for rs in range(0, STEPS, ROW)
                    for h in range(NCHAIN)
                ]
                n_ro = 0
                for r in range(STEPS):
                    for h in range(NCHAIN):
                        emit_step(h, r)
                    if probe == "noro":
                        continue
                    if n_ro < len(windows):
                        h, rs = windows[n_ro]
                        if rs + ROW <= r:  # steps of the window are done
                            emit_ro(h, rs, n_ro % 2 == 0)
                            n_ro += 1
                if probe != "noro":
                    while n_ro < len(windows):
                        h, rs = windows[n_ro]
                        emit_ro(h, rs, n_ro % 2 == 0)
                        n_ro += 1

            if reps == 1:
                scan_body()
            else:
                with tc_ctx.For_i(0, reps, 1) as _i:
                    scan_body(_i)

    nc.compile()
    return nc


def host_prep(x, W_in, b_in, W_res, b_res, W_ro, t_total=T):
    """Produce the per-core in_maps (host-side layout/dtype prep only)."""
    a = np.float32(LEAKY)
    AT = (a * W_res).T.astype(np.float32)                     # [in, out]
    wres = (
        AT.reshape(NCH, 128, NCH, 128).transpose(1, 0, 2, 3).reshape(128, 16 * 128)
    ).astype(ml_dtypes.bfloat16)
    win = W_in.T.astype(ml_dtypes.bfloat16)                   # [128, 512]
    R = (a * W_ro).T.astype(np.float32)                       # [512, 128]
    wro = R.reshape(NCH, 128, 128).transpose(1, 0, 2).reshape(128, NCH * 128).astype(
        ml_dtypes.bfloat16
    )
    bvec = (b_in + b_res).astype(np.float32)
    biasr = bvec.reshape(1, NCH * 128).astype(ml_dtypes.bfloat16)
    bias = bvec.reshape(NCH, 128).T.copy()                    # [128, 4]

    in_maps = []
    for c in range(NCORES):
        xt = np.empty((128, NCHAIN * STEPS * LANES), np.float32)
        xv = xt.reshape(128, NCHAIN, STEPS, 2, B)             # [d, h, r, half, b]
        for h in range(NCHAIN):
            for half in range(2):
                t0 = slot_t0(SPC * c + 2 * h + half)
                xv[:, h, :, half, :] = x[:, t0 : t0 + STEPS, :].transpose(2, 1, 0)
        in_maps.append({
            "xt": xt.astype(ml_dtypes.bfloat16),
            "wres": wres, "win": win, "wro": wro,
            "biasr": biasr, "bias": bias,
        })
    return in_maps


_nc_cache = {}


def kernel(x, W_in, b_in, W_res, b_res, W_ro):
    """Full inputs in, full output out ([B, T, D] float32)."""
    global _last_results
    x, W_in, b_in, W_res, b_res, W_ro = (
        np.asarray(t, dtype=np.float32) for t in (x, W_in, b_in, W_res, b_res, W_ro)
    )
    t_total = x.shape[1]
    if t_total not in _nc_cache:
        _nc_cache[t_total] = build(t_total=t_total)
    nc = _nc_cache[t_total]

    in_maps = host_prep(x, W_in, b_in, W_res, b_res, W_ro, t_total=t_total)
    res = run_bass_kernel_spmd(nc, in_maps, list(range(NCORES)), trace=TRACE)
    _last_results = res

    out = np.empty((B, t_total, D), dtype=np.float32)
    for c in range(NCORES):
        oc = res.results[c]["out"].reshape(128, NCHAIN, STEPS, 2, B)
        for h in range(NCHAIN):
            for half in range(2):
                s = SPC * c + 2 * h + half
                t0 = slot_t0(s)
                u0 = 0 if s == 0 else WO
                out[:, t0 + u0 : t0 + u0 + SEGLEN, :] = (
                    oc[:, h, u0 : u0 + SEGLEN, half, :].transpose(2, 1, 0)
                )
    return out


# revision 13
# speedup vs baseline: 3.1523x; 1.1340x over previous
"""AdaptiveESN Trainium2 kernel — dual fused-chain sequence-split (v4).

Echo State Network: B=64, T=2048, D=128, H=512, leaky a=0.26.
    h_t = (1-a) h_{t-1} + a tanh(x_t W_in^T + b_in + h_{t-1} W_res^T + b_res)
    y_t = h_t W_ro^T

The map is strongly contracting (state error decays ~0.74x/step), so a
chain restarted from h=0 converges to the true trajectory in ~32 steps.

Strategy: 32 overlapping slots of 80 steps (64 useful + 16 washout;
slot 0 starts at t=0 where h=0 is exact, so all its outputs are valid).
Core c runs TWO independent fused 128-lane chains (chain h = slots
4c+2h, 4c+2h+1), interleaved round-robin so one chain's matmuls hide the
other's cross-engine (PE->ACT->DVE->PE) epilogue latency. Per chain-step:
20 matmuls of 128 cols (16 W_res tiles as stationary + 4 W_in), with the
j=3 contraction chunk deferred last in each accumulation group; then 4
per-chunk tanh on ACT (bias via per-partition ACT bias) and 4 fused AXPY
blends on DVE (h' = (1-a) h + p; state h~ = h/a with a folded into
W_res/W_ro so the blend is one scalar_tensor_tensor). Readout (4 steps x
128 lanes per window) is spread ~one window per round between scan steps.
The dominant cost on this part is ~50 ns of sync/dispatch overhead per
instruction, so everything is shaped to minimize instruction count at
maximum tile width; PSUM accumulation groups must stay sequential per
region (interleaving groups on one PSUM tile mis-accumulates).

Layouts (host-prepped, per core c; chain h covers slots s=4c+2h (lanes
0-63) and s+1 (lanes 64-127); t_s = 0 for s=0 else 64 s - 16):
    xt   bf16 [128, 2*80*128]  xt[d, (h*80+r)*128+half*64+b] = x[b, t_s+r, d]
    wres bf16 [128, 2048]      tile (j,i) at cols (j*4+i)*128: (a W_res).T block
    win  bf16 [128, 512]       W_in.T
    wro  bf16 [128, 512]       tile j at cols j*128: (a W_ro).T block
    bias f32  [128, 4]         (b_in + b_res) chunk i in col i
    out  f32  [128, 2*80*128]  out[d, (h*80+r)*128+half*64+b] = y[b, t_s+r, d]
Host keeps steps [0,64) of slot 0 and [16,80) of slots s>=1.
"""
import sys

if "/opt/trn_rl_repo" not in sys.path:
    sys.path.insert(0, "/opt/trn_rl_repo")

import numpy as np
import ml_dtypes

import concourse.bass as bass
from concourse import bacc
import concourse.mybir as mybir
import concourse.tile as tile
from concourse.bass_utils import run_bass_kernel_spmd

try:
    import jax

    jax.config.update("jax_compilation_cache_dir", "/tmp/jax_neff_cache")
    jax.config.update("jax_persistent_cache_min_compile_time_secs", 10)
except Exception:
    pass

B, T, D, H = 64, 2048, 128, 512
LEAKY = 0.26
NCORES = 8
NCH = H // 128            # H chunks (partition tiles)
SPC = 4                   # slots per core
NCHAIN = SPC // 2         # fused 128-lane chains per core
NSLOT = NCORES * SPC      # global slots
SEGLEN = T // NSLOT       # stride between slot starts (64)
WO = 16                   # discarded washout steps for slots >= 1
STEPS = SEGLEN + WO       # chain length (80)
LANES = 128               # lanes per fused chain (2 slots x 64 batch)
WST = NCH * LANES         # state cols per step (512)
TCB = 8                   # steps per state buffer
NBUF = 3                  # state buffers per chain
ROW = 4                   # steps per readout window (4*128 = 512 cols)
BF16 = mybir.dt.bfloat16
F32 = mybir.dt.float32

TRACE = False
_last_results = None


def slot_t0(s):
    return 0 if s == 0 else SEGLEN * s - WO


def build(t_total=T, tc=TCB, reps=1, probe=None, fat=False, fatdve=False, rodma=False):
    """Build the per-core Bacc graph (same graph on all 8 cores).

    reps > 1 wraps the scan in a hardware For_i loop for wall-clock delta
    timing (per-scan = (wall_hi - wall_lo) / (reps_hi - reps_lo)).

    probe: timing-only structural variants (WRONG math, never for output):
      "zrhs"  - scan matmuls read h0 (zero) instead of hprev
      "noro"  - skip readout matmuls/copies/DMAs
      "nodve" - ACT writes states directly (no blend)
    fat=False: per-chunk ACT(+bias)/DVE epilogue (no bias matmuls).
    """
    assert t_total == T, "slot layout is hardcoded for T=2048"
    nc = bacc.Bacc(None, target_bir_lowering=False)
    xt_e = nc.declare_dram_parameter("xt", [128, NCHAIN * STEPS * LANES], BF16, isOutput=False)
    wres_e = nc.declare_dram_parameter("wres", [128, 16 * 128], BF16, isOutput=False)
    win_e = nc.declare_dram_parameter("win", [128, NCH * 128], BF16, isOutput=False)
    wro_e = nc.declare_dram_parameter("wro", [128, NCH * 128], BF16, isOutput=False)
    biasr_e = nc.declare_dram_parameter("biasr", [1, NCH * 128], BF16, isOutput=False)
    bias_e = nc.declare_dram_parameter("bias", [128, NCH], F32, isOutput=False)
    out_e = nc.declare_dram_parameter("out", [128, NCHAIN * STEPS * LANES], F32, isOutput=True)

    with tile.TileContext(nc) as tc_ctx:
        with (
            tc_ctx.tile_pool(name="const", bufs=1) as const_pool,
            tc_ctx.tile_pool(name="p", bufs=6) as p_pool,
            tc_ctx.tile_pool(name="ostage", bufs=3) as o_pool,
            tc_ctx.tile_pool(name="scan_ps", bufs=4, space=bass.MemorySpace.PSUM) as ps_pool,
            tc_ctx.tile_pool(name="ro_ps", bufs=2, space=bass.MemorySpace.PSUM) as ro_pool,
        ):
            xt_sb = const_pool.tile([128, NCHAIN * STEPS * LANES], BF16)
            wres_sb = const_pool.tile([128, 16 * 128], BF16)
            win_sb = const_pool.tile([128, NCH * 128], BF16)
            wro_sb = const_pool.tile([128, NCH * 128], BF16)
            biasr_sb = const_pool.tile([1, NCH * 128], BF16)
            bias_sb = const_pool.tile([128, NCH], F32)
            ones_sb = const_pool.tile([1, LANES], BF16)
            h0_sb = const_pool.tile([128, WST], BF16)
            # states per chain, step-major: col (r%TCB)*WST + i*LANES + lane
            st = [
                [
                    const_pool.tile([128, TCB * WST], BF16, name=f"st{h}_{n}", tag=f"st{h}_{n}")
                    for n in range(NBUF)
                ]
                for h in range(NCHAIN)
            ]

            nc.sync.dma_start(wres_sb[:], wres_e[:])
            nc.sync.dma_start(win_sb[:], win_e[:])
            nc.sync.dma_start(wro_sb[:], wro_e[:])
            nc.sync.dma_start(biasr_sb[:], biasr_e[:])
            nc.sync.dma_start(bias_sb[:], bias_e[:])
            nc.sync.dma_start(xt_sb[:], xt_e[:])
            nc.vector.memset(ones_sb[:], 1.0)
            nc.vector.memset(h0_sb[:], 0.0)

            def emit_step(h, r):
                if r == 0:
                    hprev = h0_sb[:]
                else:
                    bprev = ((r - 1) // TCB) % NBUF
                    sprev = (r - 1) % TCB
                    hprev = st[h][bprev][:, sprev * WST : (sprev + 1) * WST]
                bcur = (r // TCB) % NBUF
                scur = r % TCB
                xcol = xt_sb[:, (h * STEPS + r) * LANES : (h * STEPS + r + 1) * LANES]
                hsrc = h0_sb[:] if probe == "zrhs" else hprev

                ps = ps_pool.tile([128, WST], F32)

                def psw(i):
                    return ps[:, i * LANES : (i + 1) * LANES]

                def hcol(j):
                    return hsrc[:, j * LANES : (j + 1) * LANES]

                # per-region accumulation groups stay sequential (interleaved
                # groups on one PSUM tile mis-accumulate); (bias, win) first
                # have no state dependency, j=3 deferred last.
                for i in range(NCH):
                    ops = [(win_sb[:, i * 128 : (i + 1) * 128], xcol)]
                    if fat:
                        ops.insert(0, (biasr_sb[:, i * 128 : (i + 1) * 128], ones_sb[:]))
                    ops += [
                        (wres_sb[:, (j * NCH + i) * 128 : (j * NCH + i + 1) * 128], hcol(j))
                        for j in range(NCH)
                    ]
                    for kk, (lhsT, rhs) in enumerate(ops):
                        nc.tensor.matmul(
                            psw(i), lhsT, rhs,
                            start=(kk == 0), stop=(kk == len(ops) - 1))

                st_step = st[h][bcur][:, scur * WST : (scur + 1) * WST]
                if fat:
                    if probe == "nodve":
                        nc.scalar.activation(
                            st_step, ps[:], mybir.ActivationFunctionType.Tanh)
                    else:
                        p_t = p_pool.tile([128, WST], BF16)
                        nc.scalar.activation(
                            p_t[:], ps[:], mybir.ActivationFunctionType.Tanh)
                        nc.vector.scalar_tensor_tensor(
                            st_step, hprev, 1.0 - LEAKY, p_t[:],
                            op0=mybir.AluOpType.mult, op1=mybir.AluOpType.add)
                elif fatdve:
                    # 4 thin tanh (per-chunk bias) into one p tile, 1 AXPY
                    p_t = p_pool.tile([128, WST], BF16)
                    for i in range(NCH):
                        nc.scalar.activation(
                            p_t[:, i * LANES : (i + 1) * LANES], psw(i),
                            mybir.ActivationFunctionType.Tanh,
                            bias=bias_sb[:, i : i + 1])
                    nc.vector.scalar_tensor_tensor(
                        st_step, hprev, 1.0 - LEAKY, p_t[:],
                        op0=mybir.AluOpType.mult, op1=mybir.AluOpType.add)
                else:
                    for i in range(NCH):
                        st_col = st_step[:, i * LANES : (i + 1) * LANES]
                        bias_ap = bias_sb[:, i : i + 1]
                        if probe == "nodve":
                            nc.scalar.activation(
                                st_col, psw(i), mybir.ActivationFunctionType.Tanh,
                                bias=bias_ap)
                        else:
                            p_t = p_pool.tile([128, LANES], BF16)
                            nc.scalar.activation(
                                p_t[:], psw(i), mybir.ActivationFunctionType.Tanh,
                                bias=bias_ap)
                            nc.vector.scalar_tensor_tensor(
                                st_col,
                                hprev[:, i * LANES : (i + 1) * LANES],
                                1.0 - LEAKY, p_t[:],
                                op0=mybir.AluOpType.mult, op1=mybir.AluOpType.add)

            def emit_ro(h, rs, alt):
                # readout of chain h states for steps [rs, rs+ROW), 128 lanes
                b = (rs // TCB) % NBUF
                ls = rs % TCB
                st_v = st[h][b].rearrange("p (s w) -> p s w", w=WST)
                rps = ro_pool.tile([128, ROW * LANES], F32)
                for j in range(NCH):
                    nc.tensor.matmul(
                        rps[:],
                        wro_sb[:, j * 128 : (j + 1) * 128],
                        st_v[:, ls : ls + ROW, j * LANES : (j + 1) * LANES],
                        start=(j == 0),
                        stop=(j == NCH - 1),
                    )
                dst = out_e[:, (h * STEPS + rs) * LANES : (h * STEPS + rs + ROW) * LANES]
                if rodma:
                    nc.sync.dma_start(dst, rps[:])
                else:
                    ostage = o_pool.tile([128, ROW * LANES], F32)
                    if alt:
                        nc.scalar.copy(ostage[:], rps[:])
                    else:
                        nc.vector.tensor_copy(ostage[:], rps[:])
                    nc.sync.dma_start(dst, ostage[:])

            def scan_body(_iv=None):
                # windows in production order; one emitted per round
                windows = [
                    (h, rs)
                    for rs in range(0, STEPS, ROW)
                    for h in range(NCHAIN)
                ]
                n_ro = 0
                for r in range(STEPS):
                    for h in range(NCHAIN):
                        emit_step(h, r)
                    if probe == "noro":
                        continue
                    if n_ro < len(windows):
                        h, rs = windows[n_ro]
                        if rs + ROW <= r:  # steps of the window are done
                            emit_ro(h, rs, n_ro % 2 == 0)
                            n_ro += 1
                if probe != "noro":
                    while n_ro < len(windows):
                        h, rs = windows[n_ro]
                        emit_ro(h, rs, n_ro % 2 == 0)
                        n_ro += 1

            if reps == 1:
                scan_body()
            else:
                with tc_ctx.For_i(0, reps, 1) as _i:
                    scan_body(_i)

    nc.compile()
    return nc


def host_prep(x, W_in, b_in, W_res, b_res, W_ro, t_total=T):
    """Produce the per-core in_maps (host-side layout/dtype prep only)."""
    a = np.float32(LEAKY)
    AT = (a * W_res).T.astype(np.float32)                     # [in, out]
    wres = (
        AT.reshape(NCH, 128, NCH, 128).transpose(1, 0, 2, 3).reshape(128, 16 * 128)
    ).astype(ml_dtypes.bfloat16)
    win = W_in.T.astype(ml_dtypes.bfloat16)                   # [128, 512]
    R = (a * W_ro).T.astype(np.float32)                       # [512, 128]
    wro = R.reshape(NCH, 128, 128).transpose(1, 0, 2).reshape(128, NCH * 128).astype(
        ml_dtypes.bfloat16
    )
    bvec = (b_in + b_res).astype(np.float32)
    biasr = bvec.reshape(1, NCH * 128).astype(ml_dtypes.bfloat16)
    bias = bvec.reshape(NCH, 128).T.copy()                    # [128, 4]

    in_maps = []
    for c in range(NCORES):
        xt = np.empty((128, NCHAIN * STEPS * LANES), np.float32)
        xv = xt.reshape(128, NCHAIN, STEPS, 2, B)             # [d, h, r, half, b]
        for h in range(NCHAIN):
            for half in range(2):
                t0 = slot_t0(SPC * c + 2 * h + half)
                xv[:, h, :, half, :] = x[:, t0 : t0 + STEPS, :].transpose(2, 1, 0)
        in_maps.append({
            "xt": xt.astype(ml_dtypes.bfloat16),
            "wres": wres, "win": win, "wro": wro,
            "biasr": biasr, "bias": bias,
        })
    return in_maps


_nc_cache = {}


def kernel(x, W_in, b_in, W_res, b_res, W_ro):
    """Full inputs in, full output out ([B, T, D] float32)."""
    global _last_results
    x, W_in, b_in, W_res, b_res, W_ro = (
        np.asarray(t, dtype=np.float32) for t in (x, W_in, b_in, W_res, b_res, W_ro)
    )
    t_total = x.shape[1]
    if t_total not in _nc_cache:
        _nc_cache[t_total] = build(t_total=t_total)
    nc = _nc_cache[t_total]

    in_maps = host_prep(x, W_in, b_in, W_res, b_res, W_ro, t_total=t_total)
    res = run_bass_kernel_spmd(nc, in_maps, list(range(NCORES)), trace=TRACE)
    _last_results = res

    out = np.empty((B, t_total, D), dtype=np.float32)
    for c in range(NCORES):
        oc = res.results[c]["out"].reshape(128, NCHAIN, STEPS, 2, B)
        for h in range(NCHAIN):
            for half in range(2):
                s = SPC * c + 2 * h + half
                t0 = slot_t0(s)
                u0 = 0 if s == 0 else WO
                out[:, t0 + u0 : t0 + u0 + SEGLEN, :] = (
                    oc[:, h, u0 : u0 + SEGLEN, half, :].transpose(2, 1, 0)
                )
    return out


# revision 15
# speedup vs baseline: 3.5851x; 1.1373x over previous
"""AdaptiveESN Trainium2 kernel — dual fused-chain sequence-split (v4).

Echo State Network: B=64, T=2048, D=128, H=512, leaky a=0.26.
    h_t = (1-a) h_{t-1} + a tanh(x_t W_in^T + b_in + h_{t-1} W_res^T + b_res)
    y_t = h_t W_ro^T

The map is strongly contracting (state error decays ~0.74x/step), so a
chain restarted from h=0 converges to the true trajectory in ~32 steps.

Strategy: 32 overlapping slots of 80 steps (64 useful + 16 washout;
slot 0 starts at t=0 where h=0 is exact, so all its outputs are valid).
Core c runs TWO independent fused 128-lane chains (chain h = slots
4c+2h, 4c+2h+1), interleaved round-robin so one chain's matmuls hide the
other's cross-engine (PE->ACT->DVE->PE) epilogue latency. Per chain-step:
20 matmuls of 128 cols (16 W_res tiles as stationary + 4 W_in), with the
j=3 contraction chunk deferred last in each accumulation group; then 4
per-chunk tanh on ACT (bias via per-partition ACT bias) and 4 fused AXPY
blends on DVE (h' = (1-a) h + p; state h~ = h/a with a folded into
W_res/W_ro so the blend is one scalar_tensor_tensor). Readout (4 steps x
128 lanes per window) is spread ~one window per round between scan steps.
The dominant cost on this part is ~50 ns of sync/dispatch overhead per
instruction, so everything is shaped to minimize instruction count at
maximum tile width; PSUM accumulation groups must stay sequential per
region (interleaving groups on one PSUM tile mis-accumulates).

Layouts (host-prepped, per core c; chain h covers slots s=4c+2h (lanes
0-63) and s+1 (lanes 64-127); t_s = 0 for s=0 else 64 s - 16):
    xt   bf16 [128, 2*80*128]  xt[d, (h*80+r)*128+half*64+b] = x[b, t_s+r, d]
    wres bf16 [128, 2048]      tile (j,i) at cols (j*4+i)*128: (a W_res).T block
    win  bf16 [128, 512]       W_in.T
    wro  bf16 [128, 512]       tile j at cols j*128: (a W_ro).T block
    bias f32  [128, 4]         (b_in + b_res) chunk i in col i
    out  f32  [128, 2*80*128]  out[d, (h*80+r)*128+half*64+b] = y[b, t_s+r, d]
Host keeps steps [0,64) of slot 0 and [16,80) of slots s>=1.
"""
import sys

if "/opt/trn_rl_repo" not in sys.path:
    sys.path.insert(0, "/opt/trn_rl_repo")

import numpy as np
import ml_dtypes

import concourse.bass as bass
from concourse import bacc
import concourse.mybir as mybir
import concourse.tile as tile
from concourse.bass_utils import run_bass_kernel_spmd

try:
    import jax

    jax.config.update("jax_compilation_cache_dir", "/tmp/jax_neff_cache")
    jax.config.update("jax_persistent_cache_min_compile_time_secs", 10)
except Exception:
    pass

B, T, D, H = 64, 2048, 128, 512
LEAKY = 0.26
NCORES = 8
NCH = H // 128            # H chunks (partition tiles)
SPC = 4                   # slots per core
NCHAIN = SPC // 2         # fused 128-lane chains per core
NSLOT = NCORES * SPC      # global slots
SEGLEN = T // NSLOT       # stride between slot starts (64)
WO = 16                   # discarded washout steps for slots >= 1
STEPS = SEGLEN + WO       # chain length (80)
LANES = 128               # lanes per fused chain (2 slots x 64 batch)
WST = NCH * LANES         # state cols per step (512)
TCB = 8                   # steps per state buffer
NBUF = 3                  # state buffers per chain
ROW = 4                   # steps per readout window (4*128 = 512 cols)
BF16 = mybir.dt.bfloat16
F32 = mybir.dt.float32

TRACE = False
_last_results = None


def slot_t0(s):
    return 0 if s == 0 else SEGLEN * s - WO


def build(t_total=T, tc=TCB, reps=1, probe=None, fat=False, fatdve=False, rodma=False, psb=6, rob=2):
    """Build the per-core Bacc graph (same graph on all 8 cores).

    reps > 1 wraps the scan in a hardware For_i loop for wall-clock delta
    timing (per-scan = (wall_hi - wall_lo) / (reps_hi - reps_lo)).

    probe: timing-only structural variants (WRONG math, never for output):
      "zrhs"  - scan matmuls read h0 (zero) instead of hprev
      "noro"  - skip readout matmuls/copies/DMAs
      "nodve" - ACT writes states directly (no blend)
    fat=False: per-chunk ACT(+bias)/DVE epilogue (no bias matmuls).
    """
    assert t_total == T, "slot layout is hardcoded for T=2048"
    nc = bacc.Bacc(None, target_bir_lowering=False)
    xt_e = nc.declare_dram_parameter("xt", [128, NCHAIN * STEPS * LANES], BF16, isOutput=False)
    wres_e = nc.declare_dram_parameter("wres", [128, 16 * 128], BF16, isOutput=False)
    win_e = nc.declare_dram_parameter("win", [128, NCH * 128], BF16, isOutput=False)
    wro_e = nc.declare_dram_parameter("wro", [128, NCH * 128], BF16, isOutput=False)
    biasr_e = nc.declare_dram_parameter("biasr", [1, NCH * 128], BF16, isOutput=False)
    bias_e = nc.declare_dram_parameter("bias", [128, NCH], F32, isOutput=False)
    out_e = nc.declare_dram_parameter("out", [128, NCHAIN * STEPS * LANES], F32, isOutput=True)

    with tile.TileContext(nc) as tc_ctx:
        with (
            tc_ctx.tile_pool(name="const", bufs=1) as const_pool,
            tc_ctx.tile_pool(name="p", bufs=6) as p_pool,
            tc_ctx.tile_pool(name="ostage", bufs=3) as o_pool,
            tc_ctx.tile_pool(name="scan_ps", bufs=psb, space=bass.MemorySpace.PSUM) as ps_pool,
            tc_ctx.tile_pool(name="ro_ps", bufs=rob, space=bass.MemorySpace.PSUM) as ro_pool,
        ):
            xt_sb = const_pool.tile([128, NCHAIN * STEPS * LANES], BF16)
            wres_sb = const_pool.tile([128, 16 * 128], BF16)
            win_sb = const_pool.tile([128, NCH * 128], BF16)
            wro_sb = const_pool.tile([128, NCH * 128], BF16)
            biasr_sb = const_pool.tile([1, NCH * 128], BF16)
            bias_sb = const_pool.tile([128, NCH], F32)
            ones_sb = const_pool.tile([1, LANES], BF16)
            h0_sb = const_pool.tile([128, WST], BF16)
            # states per chain, step-major: col (r%TCB)*WST + i*LANES + lane
            st = [
                [
                    const_pool.tile([128, TCB * WST], BF16, name=f"st{h}_{n}", tag=f"st{h}_{n}")
                    for n in range(NBUF)
                ]
                for h in range(NCHAIN)
            ]

            nc.sync.dma_start(wres_sb[:], wres_e[:])
            nc.sync.dma_start(win_sb[:], win_e[:])
            nc.sync.dma_start(wro_sb[:], wro_e[:])
            nc.sync.dma_start(biasr_sb[:], biasr_e[:])
            nc.sync.dma_start(bias_sb[:], bias_e[:])
            nc.sync.dma_start(xt_sb[:], xt_e[:])
            nc.vector.memset(ones_sb[:], 1.0)
            nc.vector.memset(h0_sb[:], 0.0)

            def emit_step(h, r):
                if r == 0:
                    hprev = h0_sb[:]
                else:
                    bprev = ((r - 1) // TCB) % NBUF
                    sprev = (r - 1) % TCB
                    hprev = st[h][bprev][:, sprev * WST : (sprev + 1) * WST]
                bcur = (r // TCB) % NBUF
                scur = r % TCB
                xcol = xt_sb[:, (h * STEPS + r) * LANES : (h * STEPS + r + 1) * LANES]
                hsrc = h0_sb[:] if probe == "zrhs" else hprev

                ps = ps_pool.tile([128, WST], F32)

                def psw(i):
                    return ps[:, i * LANES : (i + 1) * LANES]

                def hcol(j):
                    return hsrc[:, j * LANES : (j + 1) * LANES]

                # per-region accumulation groups stay sequential (interleaved
                # groups on one PSUM tile mis-accumulate); (bias, win) first
                # have no state dependency, j=3 deferred last.
                for i in range(NCH):
                    ops = [(win_sb[:, i * 128 : (i + 1) * 128], xcol)]
                    if fat:
                        ops.insert(0, (biasr_sb[:, i * 128 : (i + 1) * 128], ones_sb[:]))
                    ops += [
                        (wres_sb[:, (j * NCH + i) * 128 : (j * NCH + i + 1) * 128], hcol(j))
                        for j in range(NCH)
                    ]
                    for kk, (lhsT, rhs) in enumerate(ops):
                        nc.tensor.matmul(
                            psw(i), lhsT, rhs,
                            start=(kk == 0), stop=(kk == len(ops) - 1))

                st_step = st[h][bcur][:, scur * WST : (scur + 1) * WST]
                if fat:
                    if probe == "nodve":
                        nc.scalar.activation(
                            st_step, ps[:], mybir.ActivationFunctionType.Tanh)
                    else:
                        p_t = p_pool.tile([128, WST], BF16)
                        nc.scalar.activation(
                            p_t[:], ps[:], mybir.ActivationFunctionType.Tanh)
                        nc.vector.scalar_tensor_tensor(
                            st_step, hprev, 1.0 - LEAKY, p_t[:],
                            op0=mybir.AluOpType.mult, op1=mybir.AluOpType.add)
                elif fatdve:
                    # 4 thin tanh (per-chunk bias) into one p tile, 1 AXPY
                    p_t = p_pool.tile([128, WST], BF16)
                    for i in range(NCH):
                        nc.scalar.activation(
                            p_t[:, i * LANES : (i + 1) * LANES], psw(i),
                            mybir.ActivationFunctionType.Tanh,
                            bias=bias_sb[:, i : i + 1])
                    nc.vector.scalar_tensor_tensor(
                        st_step, hprev, 1.0 - LEAKY, p_t[:],
                        op0=mybir.AluOpType.mult, op1=mybir.AluOpType.add)
                else:
                    for i in range(NCH):
                        st_col = st_step[:, i * LANES : (i + 1) * LANES]
                        bias_ap = bias_sb[:, i : i + 1]
                        if probe == "nodve":
                            nc.scalar.activation(
                                st_col, psw(i), mybir.ActivationFunctionType.Tanh,
                                bias=bias_ap)
                        else:
                            p_t = p_pool.tile([128, LANES], BF16)
                            nc.scalar.activation(
                                p_t[:], psw(i), mybir.ActivationFunctionType.Tanh,
                                bias=bias_ap)
                            nc.vector.scalar_tensor_tensor(
                                st_col,
                                hprev[:, i * LANES : (i + 1) * LANES],
                                1.0 - LEAKY, p_t[:],
                                op0=mybir.AluOpType.mult, op1=mybir.AluOpType.add)

            def emit_ro(h, rs, alt):
                # readout of chain h states for steps [rs, rs+ROW), 128 lanes
                b = (rs // TCB) % NBUF
                ls = rs % TCB
                st_v = st[h][b].rearrange("p (s w) -> p s w", w=WST)
                rps = ro_pool.tile([128, ROW * LANES], F32)
                for j in range(NCH):
                    nc.tensor.matmul(
                        rps[:],
                        wro_sb[:, j * 128 : (j + 1) * 128],
                        st_v[:, ls : ls + ROW, j * LANES : (j + 1) * LANES],
                        start=(j == 0),
                        stop=(j == NCH - 1),
                    )
                dst = out_e[:, (h * STEPS + rs) * LANES : (h * STEPS + rs + ROW) * LANES]
                if rodma:
                    nc.sync.dma_start(dst, rps[:])
                else:
                    ostage = o_pool.tile([128, ROW * LANES], F32)
                    if alt:
                        nc.scalar.copy(ostage[:], rps[:])
                    else:
                        nc.vector.tensor_copy(ostage[:], rps[:])
                    nc.sync.dma_start(dst, ostage[:])

            def scan_body(_iv=None):
                # windows in production order; one emitted per round
                windows = [
                    (h, rs)
                    for rs in range(0, STEPS, ROW)
                    for h in range(NCHAIN)
                ]
                n_ro = 0
                for r in range(STEPS):
                    for h in range(NCHAIN):
                        emit_step(h, r)
                    if probe == "noro":
                        continue
                    if n_ro < len(windows):
                        h, rs = windows[n_ro]
                        if rs + ROW <= r:  # steps of the window are done
                            emit_ro(h, rs, n_ro % 2 == 0)
                            n_ro += 1
                if probe != "noro":
                    while n_ro < len(windows):
                        h, rs = windows[n_ro]
                        emit_ro(h, rs, n_ro % 2 == 0)
                        n_ro += 1

            if reps == 1:
                scan_body()
            else:
                with tc_ctx.For_i(0, reps, 1) as _i:
                    scan_body(_i)

    nc.compile()
    return nc


def host_prep(x, W_in, b_in, W_res, b_res, W_ro, t_total=T):
    """Produce the per-core in_maps (host-side layout/dtype prep only)."""
    a = np.float32(LEAKY)
    AT = (a * W_res).T.astype(np.float32)                     # [in, out]
    wres = (
        AT.reshape(NCH, 128, NCH, 128).transpose(1, 0, 2, 3).reshape(128, 16 * 128)
    ).astype(ml_dtypes.bfloat16)
    win = W_in.T.astype(ml_dtypes.bfloat16)                   # [128, 512]
    R = (a * W_ro).T.astype(np.float32)                       # [512, 128]
    wro = R.reshape(NCH, 128, 128).transpose(1, 0, 2).reshape(128, NCH * 128).astype(
        ml_dtypes.bfloat16
    )
    bvec = (b_in + b_res).astype(np.float32)
    biasr = bvec.reshape(1, NCH * 128).astype(ml_dtypes.bfloat16)
    bias = bvec.reshape(NCH, 128).T.copy()                    # [128, 4]

    in_maps = []
    for c in range(NCORES):
        xt = np.empty((128, NCHAIN * STEPS * LANES), np.float32)
        xv = xt.reshape(128, NCHAIN, STEPS, 2, B)             # [d, h, r, half, b]
        for h in range(NCHAIN):
            for half in range(2):
                t0 = slot_t0(SPC * c + 2 * h + half)
                xv[:, h, :, half, :] = x[:, t0 : t0 + STEPS, :].transpose(2, 1, 0)
        in_maps.append({
            "xt": xt.astype(ml_dtypes.bfloat16),
            "wres": wres, "win": win, "wro": wro,
            "biasr": biasr, "bias": bias,
        })
    return in_maps


_nc_cache = {}


def kernel(x, W_in, b_in, W_res, b_res, W_ro):
    """Full inputs in, full output out ([B, T, D] float32)."""
    global _last_results
    x, W_in, b_in, W_res, b_res, W_ro = (
        np.asarray(t, dtype=np.float32) for t in (x, W_in, b_in, W_res, b_res, W_ro)
    )
    t_total = x.shape[1]
    if t_total not in _nc_cache:
        _nc_cache[t_total] = build(t_total=t_total)
    nc = _nc_cache[t_total]

    in_maps = host_prep(x, W_in, b_in, W_res, b_res, W_ro, t_total=t_total)
    res = run_bass_kernel_spmd(nc, in_maps, list(range(NCORES)), trace=TRACE)
    _last_results = res

    out = np.empty((B, t_total, D), dtype=np.float32)
    for c in range(NCORES):
        oc = res.results[c]["out"].reshape(128, NCHAIN, STEPS, 2, B)
        for h in range(NCHAIN):
            for half in range(2):
                s = SPC * c + 2 * h + half
                t0 = slot_t0(s)
                u0 = 0 if s == 0 else WO
                out[:, t0 + u0 : t0 + u0 + SEGLEN, :] = (
                    oc[:, h, u0 : u0 + SEGLEN, half, :].transpose(2, 1, 0)
                )
    return out


# revision 18
# speedup vs baseline: 3.6794x; 1.0263x over previous
"""AdaptiveESN Trainium2 kernel — dual fused-chain sequence-split (v4).

Echo State Network: B=64, T=2048, D=128, H=512, leaky a=0.26.
    h_t = (1-a) h_{t-1} + a tanh(x_t W_in^T + b_in + h_{t-1} W_res^T + b_res)
    y_t = h_t W_ro^T

The map is strongly contracting (state error decays ~0.74x/step), so a
chain restarted from h=0 converges to the true trajectory in ~32 steps.

Strategy: 32 overlapping slots of 80 steps (64 useful + 16 washout;
slot 0 starts at t=0 where h=0 is exact, so all its outputs are valid).
Core c runs TWO independent fused 128-lane chains (chain h = slots
4c+2h, 4c+2h+1), interleaved round-robin so one chain's matmuls hide the
other's cross-engine (PE->ACT->DVE->PE) epilogue latency. Per chain-step:
20 matmuls of 128 cols (16 W_res tiles as stationary + 4 W_in), with the
j=3 contraction chunk deferred last in each accumulation group; then 4
per-chunk tanh on ACT (bias via per-partition ACT bias) and 4 fused AXPY
blends on DVE (h' = (1-a) h + p; state h~ = h/a with a folded into
W_res/W_ro so the blend is one scalar_tensor_tensor). Readout (4 steps x
128 lanes per window) is spread ~one window per round between scan steps;
its PSUM->SBUF staging copies all run on DVE (ACT is the busier engine).
The dominant cost on this part is ~50 ns of sync/dispatch overhead per
instruction, so everything is shaped to minimize instruction count at
maximum tile width; PSUM accumulation groups must stay sequential per
region (interleaving groups on one PSUM tile mis-accumulates).

Layouts (host-prepped, per core c; chain h covers slots s=4c+2h (lanes
0-63) and s+1 (lanes 64-127); t_s = 0 for s=0 else 64 s - 16):
    xt   bf16 [128, 2*80*128]  xt[d, (h*80+r)*128+half*64+b] = x[b, t_s+r, d]
    wres bf16 [128, 2048]      tile (j,i) at cols (j*4+i)*128: (a W_res).T block
    win  bf16 [128, 512]       W_in.T
    wro  bf16 [128, 512]       tile j at cols j*128: (a W_ro).T block
    bias f32  [128, 4]         (b_in + b_res) chunk i in col i
    out  f32  [128, 2*80*128]  out[d, (h*80+r)*128+half*64+b] = y[b, t_s+r, d]
Host keeps steps [0,64) of slot 0 and [16,80) of slots s>=1.
"""
import sys

if "/opt/trn_rl_repo" not in sys.path:
    sys.path.insert(0, "/opt/trn_rl_repo")

import numpy as np
import ml_dtypes

import concourse.bass as bass
from concourse import bacc
import concourse.mybir as mybir
import concourse.tile as tile
from concourse.bass_utils import run_bass_kernel_spmd

try:
    import jax

    jax.config.update("jax_compilation_cache_dir", "/tmp/jax_neff_cache")
    jax.config.update("jax_persistent_cache_min_compile_time_secs", 10)
except Exception:
    pass

B, T, D, H = 64, 2048, 128, 512
LEAKY = 0.26
NCORES = 8
NCH = H // 128            # H chunks (partition tiles)
SPC = 4                   # slots per core
NCHAIN = SPC // 2         # fused 128-lane chains per core
NSLOT = NCORES * SPC      # global slots
SEGLEN = T // NSLOT       # stride between slot starts (64)
WO = 16                   # discarded washout steps for slots >= 1
STEPS = SEGLEN + WO       # chain length (80)
LANES = 128               # lanes per fused chain (2 slots x 64 batch)
WST = NCH * LANES         # state cols per step (512)
TCB = 8                   # steps per state buffer
NBUF = 3                  # state buffers per chain
ROW = 4                   # steps per readout window (4*128 = 512 cols)
BF16 = mybir.dt.bfloat16
F32 = mybir.dt.float32

TRACE = False
_last_results = None


def slot_t0(s):
    return 0 if s == 0 else SEGLEN * s - WO


def build(t_total=T, tc=TCB, reps=1, probe=None, fat=False, fatdve=False, rodma=False, psb=6, rob=2, pb=6, ob=3, dvepair=False, rocopy=False):
    """Build the per-core Bacc graph (same graph on all 8 cores).

    reps > 1 wraps the scan in a hardware For_i loop for wall-clock delta
    timing (per-scan = (wall_hi - wall_lo) / (reps_hi - reps_lo)).

    probe: timing-only structural variants (WRONG math, never for output):
      "zrhs"  - scan matmuls read h0 (zero) instead of hprev
      "noro"  - skip readout matmuls/copies/DMAs
      "nodve" - ACT writes states directly (no blend)
    fat=False: per-chunk ACT(+bias)/DVE epilogue (no bias matmuls).
    """
    assert t_total == T, "slot layout is hardcoded for T=2048"
    nc = bacc.Bacc(None, target_bir_lowering=False)
    xt_e = nc.declare_dram_parameter("xt", [128, NCHAIN * STEPS * LANES], BF16, isOutput=False)
    wres_e = nc.declare_dram_parameter("wres", [128, 16 * 128], BF16, isOutput=False)
    win_e = nc.declare_dram_parameter("win", [128, NCH * 128], BF16, isOutput=False)
    wro_e = nc.declare_dram_parameter("wro", [128, NCH * 128], BF16, isOutput=False)
    biasr_e = nc.declare_dram_parameter("biasr", [1, NCH * 128], BF16, isOutput=False)
    bias_e = nc.declare_dram_parameter("bias", [128, NCH], F32, isOutput=False)
    out_e = nc.declare_dram_parameter("out", [128, NCHAIN * STEPS * LANES], F32, isOutput=True)

    with tile.TileContext(nc) as tc_ctx:
        with (
            tc_ctx.tile_pool(name="const", bufs=1) as const_pool,
            tc_ctx.tile_pool(name="p", bufs=pb) as p_pool,
            tc_ctx.tile_pool(name="ostage", bufs=ob) as o_pool,
            tc_ctx.tile_pool(name="scan_ps", bufs=psb, space=bass.MemorySpace.PSUM) as ps_pool,
            tc_ctx.tile_pool(name="ro_ps", bufs=rob, space=bass.MemorySpace.PSUM) as ro_pool,
        ):
            xt_sb = const_pool.tile([128, NCHAIN * STEPS * LANES], BF16)
            wres_sb = const_pool.tile([128, 16 * 128], BF16)
            win_sb = const_pool.tile([128, NCH * 128], BF16)
            wro_sb = const_pool.tile([128, NCH * 128], BF16)
            biasr_sb = const_pool.tile([1, NCH * 128], BF16)
            bias_sb = const_pool.tile([128, NCH], F32)
            ones_sb = const_pool.tile([1, LANES], BF16)
            h0_sb = const_pool.tile([128, WST], BF16)
            # states per chain, step-major: col (r%TCB)*WST + i*LANES + lane
            st = [
                [
                    const_pool.tile([128, TCB * WST], BF16, name=f"st{h}_{n}", tag=f"st{h}_{n}")
                    for n in range(NBUF)
                ]
                for h in range(NCHAIN)
            ]

            nc.sync.dma_start(wres_sb[:], wres_e[:])
            nc.sync.dma_start(win_sb[:], win_e[:])
            nc.sync.dma_start(wro_sb[:], wro_e[:])
            nc.sync.dma_start(biasr_sb[:], biasr_e[:])
            nc.sync.dma_start(bias_sb[:], bias_e[:])
            nc.sync.dma_start(xt_sb[:], xt_e[:])
            nc.vector.memset(ones_sb[:], 1.0)
            nc.vector.memset(h0_sb[:], 0.0)

            def emit_step(h, r):
                if r == 0:
                    hprev = h0_sb[:]
                else:
                    bprev = ((r - 1) // TCB) % NBUF
                    sprev = (r - 1) % TCB
                    hprev = st[h][bprev][:, sprev * WST : (sprev + 1) * WST]
                bcur = (r // TCB) % NBUF
                scur = r % TCB
                xcol = xt_sb[:, (h * STEPS + r) * LANES : (h * STEPS + r + 1) * LANES]
                hsrc = h0_sb[:] if probe == "zrhs" else hprev

                ps = ps_pool.tile([128, WST], F32)

                def psw(i):
                    return ps[:, i * LANES : (i + 1) * LANES]

                def hcol(j):
                    return hsrc[:, j * LANES : (j + 1) * LANES]

                # per-region accumulation groups stay sequential (interleaved
                # groups on one PSUM tile mis-accumulate); (bias, win) first
                # have no state dependency, j=3 deferred last.
                for i in range(NCH):
                    ops = [(win_sb[:, i * 128 : (i + 1) * 128], xcol)]
                    if fat:
                        ops.insert(0, (biasr_sb[:, i * 128 : (i + 1) * 128], ones_sb[:]))
                    ops += [
                        (wres_sb[:, (j * NCH + i) * 128 : (j * NCH + i + 1) * 128], hcol(j))
                        for j in range(NCH)
                    ]
                    for kk, (lhsT, rhs) in enumerate(ops):
                        nc.tensor.matmul(
                            psw(i), lhsT, rhs,
                            start=(kk == 0), stop=(kk == len(ops) - 1))

                st_step = st[h][bcur][:, scur * WST : (scur + 1) * WST]
                if fat:
                    if probe == "nodve":
                        nc.scalar.activation(
                            st_step, ps[:], mybir.ActivationFunctionType.Tanh)
                    else:
                        p_t = p_pool.tile([128, WST], BF16)
                        nc.scalar.activation(
                            p_t[:], ps[:], mybir.ActivationFunctionType.Tanh)
                        nc.vector.scalar_tensor_tensor(
                            st_step, hprev, 1.0 - LEAKY, p_t[:],
                            op0=mybir.AluOpType.mult, op1=mybir.AluOpType.add)
                elif fatdve:
                    # 4 thin tanh (per-chunk bias) into one p tile, 1 AXPY
                    p_t = p_pool.tile([128, WST], BF16)
                    for i in range(NCH):
                        nc.scalar.activation(
                            p_t[:, i * LANES : (i + 1) * LANES], psw(i),
                            mybir.ActivationFunctionType.Tanh,
                            bias=bias_sb[:, i : i + 1])
                    nc.vector.scalar_tensor_tensor(
                        st_step, hprev, 1.0 - LEAKY, p_t[:],
                        op0=mybir.AluOpType.mult, op1=mybir.AluOpType.add)
                elif dvepair:
                    for pair in range(2):
                        p_t = p_pool.tile([128, 2 * LANES], BF16)
                        for k2 in range(2):
                            i = 2 * pair + k2
                            nc.scalar.activation(
                                p_t[:, k2 * LANES : (k2 + 1) * LANES], psw(i),
                                mybir.ActivationFunctionType.Tanh,
                                bias=bias_sb[:, i : i + 1])
                        nc.vector.scalar_tensor_tensor(
                            st_step[:, 2 * pair * LANES : 2 * (pair + 1) * LANES],
                            hprev[:, 2 * pair * LANES : 2 * (pair + 1) * LANES],
                            1.0 - LEAKY, p_t[:],
                            op0=mybir.AluOpType.mult, op1=mybir.AluOpType.add)
                else:
                    for i in range(NCH):
                        st_col = st_step[:, i * LANES : (i + 1) * LANES]
                        bias_ap = bias_sb[:, i : i + 1]
                        if probe == "nodve":
                            nc.scalar.activation(
                                st_col, psw(i), mybir.ActivationFunctionType.Tanh,
                                bias=bias_ap)
                        else:
                            p_t = p_pool.tile([128, LANES], BF16)
                            nc.scalar.activation(
                                p_t[:], psw(i), mybir.ActivationFunctionType.Tanh,
                                bias=bias_ap)
                            nc.vector.scalar_tensor_tensor(
                                st_col,
                                hprev[:, i * LANES : (i + 1) * LANES],
                                1.0 - LEAKY, p_t[:],
                                op0=mybir.AluOpType.mult, op1=mybir.AluOpType.add)

            def emit_ro(h, rs, alt):
                # readout of chain h states for steps [rs, rs+ROW), 128 lanes
                b = (rs // TCB) % NBUF
                ls = rs % TCB
                st_v = st[h][b].rearrange("p (s w) -> p s w", w=WST)
                rps = ro_pool.tile([128, ROW * LANES], F32)
                for j in range(NCH):
                    nc.tensor.matmul(
                        rps[:],
                        wro_sb[:, j * 128 : (j + 1) * 128],
                        st_v[:, ls : ls + ROW, j * LANES : (j + 1) * LANES],
                        start=(j == 0),
                        stop=(j == NCH - 1),
                    )
                dst = out_e[:, (h * STEPS + rs) * LANES : (h * STEPS + rs + ROW) * LANES]
                if rodma:
                    nc.sync.dma_start(dst, rps[:])
                else:
                    ostage = o_pool.tile([128, ROW * LANES], F32)
                    if rocopy is not None:
                        alt = rocopy
                    if alt:
                        nc.scalar.copy(ostage[:], rps[:])
                    else:
                        nc.vector.tensor_copy(ostage[:], rps[:])
                    nc.sync.dma_start(dst, ostage[:])

            def scan_body(_iv=None):
                # windows in production order; one emitted per round
                windows = [
                    (h, rs)
                    for rs in range(0, STEPS, ROW)
                    for h in range(NCHAIN)
                ]
                n_ro = 0
                for r in range(STEPS):
                    for h in range(NCHAIN):
                        emit_step(h, r)
                    if probe == "noro":
                        continue
                    if n_ro < len(windows):
                        h, rs = windows[n_ro]
                        if rs + ROW <= r:  # steps of the window are done
                            emit_ro(h, rs, n_ro % 2 == 0)
                            n_ro += 1
                if probe != "noro":
                    while n_ro < len(windows):
                        h, rs = windows[n_ro]
                        emit_ro(h, rs, n_ro % 2 == 0)
                        n_ro += 1

            if reps == 1:
                scan_body()
            else:
                with tc_ctx.For_i(0, reps, 1) as _i:
                    scan_body(_i)

    nc.compile()
    return nc


def host_prep(x, W_in, b_in, W_res, b_res, W_ro, t_total=T):
    """Produce the per-core in_maps (host-side layout/dtype prep only)."""
    a = np.float32(LEAKY)
    AT = (a * W_res).T.astype(np.float32)                     # [in, out]
    wres = (
        AT.reshape(NCH, 128, NCH, 128).transpose(1, 0, 2, 3).reshape(128, 16 * 128)
    ).astype(ml_dtypes.bfloat16)
    win = W_in.T.astype(ml_dtypes.bfloat16)                   # [128, 512]
    R = (a * W_ro).T.astype(np.float32)                       # [512, 128]
    wro = R.reshape(NCH, 128, 128).transpose(1, 0, 2).reshape(128, NCH * 128).astype(
        ml_dtypes.bfloat16
    )
    bvec = (b_in + b_res).astype(np.float32)
    biasr = bvec.reshape(1, NCH * 128).astype(ml_dtypes.bfloat16)
    bias = bvec.reshape(NCH, 128).T.copy()                    # [128, 4]

    in_maps = []
    for c in range(NCORES):
        xt = np.empty((128, NCHAIN * STEPS * LANES), np.float32)
        xv = xt.reshape(128, NCHAIN, STEPS, 2, B)             # [d, h, r, half, b]
        for h in range(NCHAIN):
            for half in range(2):
                t0 = slot_t0(SPC * c + 2 * h + half)
                xv[:, h, :, half, :] = x[:, t0 : t0 + STEPS, :].transpose(2, 1, 0)
        in_maps.append({
            "xt": xt.astype(ml_dtypes.bfloat16),
            "wres": wres, "win": win, "wro": wro,
            "biasr": biasr, "bias": bias,
        })
    return in_maps


_nc_cache = {}


def kernel(x, W_in, b_in, W_res, b_res, W_ro):
    """Full inputs in, full output out ([B, T, D] float32)."""
    global _last_results
    x, W_in, b_in, W_res, b_res, W_ro = (
        np.asarray(t, dtype=np.float32) for t in (x, W_in, b_in, W_res, b_res, W_ro)
    )
    t_total = x.shape[1]
    if t_total not in _nc_cache:
        _nc_cache[t_total] = build(t_total=t_total)
    nc = _nc_cache[t_total]

    in_maps = host_prep(x, W_in, b_in, W_res, b_res, W_ro, t_total=t_total)
    res = run_bass_kernel_spmd(nc, in_maps, list(range(NCORES)), trace=TRACE)
    _last_results = res

    out = np.empty((B, t_total, D), dtype=np.float32)
    for c in range(NCORES):
        oc = res.results[c]["out"].reshape(128, NCHAIN, STEPS, 2, B)
        for h in range(NCHAIN):
            for half in range(2):
                s = SPC * c + 2 * h + half
                t0 = slot_t0(s)
                u0 = 0 if s == 0 else WO
                out[:, t0 + u0 : t0 + u0 + SEGLEN, :] = (
                    oc[:, h, u0 : u0 + SEGLEN, half, :].transpose(2, 1, 0)
                )
    return out
